# revision 14
# baseline (speedup 1.0000x reference)
"""Trainium2 Bass kernel for nn_CASST (dense transformer, CTMF blocks).

Self-contained: builds the Bass program from the concrete numpy inputs,
shards batch B=128 across 8 NeuronCores (16 samples each), runs SPMD,
gathers the full [128, 16] output.

Per-core layout:
  X [128, 36, 512] f32 token-major residual: tiles 0..31 = spatial stream
  (16 samples x 256 padded rows, 226 valid: cls at row 0, patches 1..225),
  tiles 32..35 = spectral stream (16 samples x 32 padded rows, 31 valid).
  Matmul operands bf16; PSUM accumulation f32.
  Attention: scores computed transposed (keys on partitions) so softmax
  needs no transpose of the attention matrix; the row-sum comes free from
  an extra all-ones column appended to V, and the normalization is fused
  into the PSUM->SBUF copy of the per-head output.
"""
import sys

sys.path.insert(0, "/opt/trn_rl_repo")

import numpy as np
import ml_dtypes

import concourse.bass as bass
import concourse.tile as tile
from concourse import bacc
from concourse import mybir
from concourse.masks import make_identity
from concourse.bass_utils import run_bass_kernel_spmd

F32 = mybir.dt.float32
BF16 = mybir.dt.bfloat16
AF = mybir.ActivationFunctionType
ALU = mybir.AluOpType

B, BANDS, HW, DIM, NH, NCLS = 128, 30, 15, 512, 8, 16
NCORES = 8
SB = B // NCORES          # 16 samples per core
NPATCH = HW * HW          # 225
SPA_PAD, SPE_PAD = 256, 32
NT_SPA = SB * SPA_PAD // 128   # 32
NT_SPE = SB * SPE_PAD // 128   # 4
NT = NT_SPA + NT_SPE           # 36
EPS = 1e-5
HD = DIM // NH            # 64


def _bf(x):
    return np.asarray(x, dtype=np.float32).astype(ml_dtypes.bfloat16)


def _pack_kT(w_T, mdim):
    """[512, M] (contraction rows) -> [128, 4, M] (partition, k-chunk, M)."""
    k = w_T.shape[0]
    return np.ascontiguousarray(w_T.reshape(k // 128, 128, mdim).transpose(1, 0, 2))


def _ident(a, b):
    return bool(np.all(np.asarray(a) == 1) and np.all(np.asarray(b) == 0))


def prepare_weights(inp):
    w = {}
    # spatial conv + BN fold
    s_h = inp["bn_h_g"] / np.sqrt(inp["bn_h_v"] + EPS)
    b_h = inp["conv_h_b"] * s_h + inp["bn_h_b"] - inp["bn_h_m"] * s_h
    w_h = np.asarray(inp["conv_h_w"]) * np.asarray(s_h)[:, None, None, None]
    # K-tile ky: row 32*kx + b holds tap (ky,kx) band b; tile0 row 96 = bias
    w_spa = np.zeros((3, 128, DIM), np.float32)
    for ky in range(3):
        for kx in range(3):
            w_spa[ky, 32 * kx:32 * kx + 30, :] = np.asarray(w_h)[:, :, ky, kx].T
    w_spa[0, 96, :] = np.asarray(b_h)
    w["w_spa"] = _bf(w_spa)

    # spectral conv + BN fold
    s_c = inp["cnn_bn_g"] / np.sqrt(inp["cnn_bn_v"] + EPS)
    b_c = inp["cnn_conv_b"] * s_c + inp["cnn_bn_b"] - inp["cnn_bn_m"] * s_c
    w_c = np.asarray(inp["cnn_conv_w"]) * np.asarray(s_c)[:, None, None, None]
    # row 32*kx + ky holds tap (ky,kx) so border memsets stay 32-aligned
    w_cnn = np.zeros((67, 128), np.float32)
    for ky in range(3):
        for kx in range(3):
            w_cnn[32 * kx + ky, :] = w_c[:, 0, ky, kx]
    w["w_cnn"] = _bf(w_cnn)
    w["b_cnn"] = np.asarray(b_c, np.float32).reshape(128, 1)
    w["w_fc"] = _bf(np.asarray(inp["cnn_fc_w"]).T / NPATCH)   # mean folded
    w["fc_b"] = np.asarray(inp["cnn_fc_b"], np.float32)

    for i in range(2):
        qkv = np.asarray(inp["blk_qkv_w"][i])
        wq, wk, wv = qkv[:DIM], qkv[DIM:2 * DIM], qkv[2 * DIM:]
        w[f"wqk{i}"] = _bf(_pack_kT(np.concatenate([wq.T, wk.T], 1), 2 * DIM))
        w[f"wv{i}"] = _bf(_pack_kT(wv.T, DIM))
        w[f"wproj{i}"] = _bf(_pack_kT(np.asarray(inp["blk_proj_w"][i]).T, DIM))
        w[f"wfc1{i}"] = _bf(_pack_kT(np.asarray(inp["blk_fc1_w"][i]).T, DIM))
        w[f"wfc2{i}"] = _bf(_pack_kT(np.asarray(inp["blk_fc2_w"][i]).T, DIM))
    w["whead"] = _bf(_pack_kT(np.asarray(inp["head_w"]).T, NCLS))

    for k in ("blk_qkv_b", "blk_proj_b", "blk_fc1_b", "blk_fc2_b", "head_b",
              "blk_n1_w", "blk_n1_b", "blk_n2_w", "blk_n2_b",
              "norm1_w", "norm1_b", "norm2_w", "norm2_b"):
        w[k] = np.asarray(inp[k], np.float32)

    # pos_spa[p, c] = positional embedding for the token that conv-psum row p
    # of chunk c produces: chunk0 row p -> token p+1, chunk1 row p -> token 128+p
    pos_shift = np.zeros((128, 2, DIM), np.float32)
    spa_pos = np.asarray(inp["spa_pos"])[0]           # [226, 512]
    pos_shift[0:127, 0] = spa_pos[1:128]
    pos_shift[0:98, 1] = spa_pos[128:226]
    w["pos_spa"] = pos_shift
    pos_spe = np.zeros((SPE_PAD, DIM), np.float32)
    pos_spe[1:1 + BANDS] = np.asarray(inp["spe_pos"])[0, 1:1 + BANDS]
    w["pos_spe"] = np.ascontiguousarray(np.tile(pos_spe, (4, 1)))
    cls2 = np.zeros((2, DIM), np.float32)
    cls2[0] = np.asarray(inp["spa_cls"])[0, 0] + np.asarray(inp["spa_pos"])[0, 0]
    cls2[1] = np.asarray(inp["spe_cls"])[0, 0] + np.asarray(inp["spe_pos"])[0, 0]
    w["cls2"] = cls2

    w["cfg"] = dict(
        use_qkv_b=bool(np.any(w["blk_qkv_b"] != 0)),
        use_proj_b=bool(np.any(w["blk_proj_b"] != 0)),
        use_fc1_b=bool(np.any(w["blk_fc1_b"] != 0)),
        use_fc2_b=bool(np.any(w["blk_fc2_b"] != 0)),
        use_fc_b=bool(np.any(w["fc_b"] != 0)),
        use_head_b=bool(np.any(w["head_b"] != 0)),
        use_n1=not all(_ident(w["blk_n1_w"][i], w["blk_n1_b"][i]) for i in range(2)),
        use_n2=not all(_ident(w["blk_n2_w"][i], w["blk_n2_b"][i]) for i in range(2)),
        use_nf=not (_ident(w["norm1_w"], w["norm1_b"])
                    and _ident(w["norm2_w"], w["norm2_b"])),
    )
    return w


def _im2cols(xc):
    """Host im2col for one core's x shard [SB, 30, 15, 15] (f32)."""
    xp = np.pad(xc, ((0, 0), (0, 0), (1, 1), (1, 1)))
    im_spa = np.zeros((3, 128, SB * NPATCH), np.float32)
    im_spe = np.zeros((67, SB * BANDS * NPATCH), np.float32)
    for ky in range(3):
        for kx in range(3):
            win = xp[:, :, ky:ky + HW, kx:kx + HW]          # [SB,30,15,15]
            im_spa[ky, 32 * kx:32 * kx + 30, :] = (
                win.transpose(1, 0, 2, 3).reshape(BANDS, -1))
            im_spe[32 * kx + ky, :] = win.reshape(-1)
    im_spa[0, 96, :] = 1.0
    return _bf(im_spa), _bf(im_spe)


def make_in_maps(inputs, w):
    x = np.asarray(inputs["x"], np.float32)[:, 0]   # [128, 30, 15, 15]
    cfg = w["cfg"]
    base = {k: w[k] for k in
            ("w_spa", "w_cnn", "b_cnn", "w_fc", "whead", "pos_spa",
             "pos_spe", "cls2")}
    for i in range(2):
        for nm in ("wqk", "wv", "wproj", "wfc1", "wfc2"):
            base[nm + str(i)] = w[nm + str(i)]
    if cfg["use_qkv_b"]:
        base["qkv_b"] = np.ascontiguousarray(
            w["blk_qkv_b"].reshape(2, 12, 128).transpose(0, 2, 1))
    if cfg["use_proj_b"]:
        base["proj_b"] = w["blk_proj_b"]
    if cfg["use_fc1_b"]:
        base["fc1_b"] = np.ascontiguousarray(
            w["blk_fc1_b"].reshape(2, 4, 128).transpose(0, 2, 1))
    if cfg["use_fc2_b"]:
        base["fc2_b"] = w["blk_fc2_b"]
    if cfg["use_fc_b"]:
        base["fc_b"] = w["fc_b"].reshape(1, DIM)
    if cfg["use_head_b"]:
        base["head_b"] = w["head_b"].reshape(1, NCLS)
    if cfg["use_n1"]:
        base["n1_wb"] = np.ascontiguousarray(
            np.stack([w["blk_n1_w"], w["blk_n1_b"]], axis=1))
    if cfg["use_n2"]:
        base["n2_wb"] = np.ascontiguousarray(
            np.stack([w["blk_n2_w"], w["blk_n2_b"]], axis=1))
    if cfg["use_nf"]:
        base["nf_w"] = np.stack([w["norm1_w"], w["norm2_w"]])
        base["nf_b"] = np.stack([w["norm1_b"], w["norm2_b"]])
    maps = []
    for c in range(NCORES):
        m = dict(base)
        m["im_spa"], m["im_spe"] = _im2cols(x[c * SB:(c + 1) * SB])
        maps.append(m)
    return maps


# =====================================================================
def build_program(w, debug_stage=None):
    cfg = w["cfg"]
    nc = bacc.Bacc(None)
    P = {}

    def dparam(name, shape, dt):
        P[name] = nc.declare_dram_parameter(name, list(shape), dt, isOutput=False)

    dparam("im_spa", (3, 128, SB * NPATCH), BF16)
    dparam("im_spe", (67, SB * BANDS * NPATCH), BF16)
    dparam("w_spa", (3, 128, DIM), BF16)
    dparam("w_cnn", (67, 128), BF16)
    dparam("b_cnn", (128, 1), F32)
    dparam("w_fc", (128, DIM), BF16)
    for i in range(2):
        dparam(f"wqk{i}", (128, 4, 2 * DIM), BF16)
        for nm in ("wv", "wproj", "wfc1", "wfc2"):
            dparam(f"{nm}{i}", (128, 4, DIM), BF16)
    dparam("whead", (128, 8, NCLS), BF16)
    dparam("pos_spa", (128, 2, DIM), F32)
    dparam("pos_spe", (128, DIM), F32)
    dparam("cls2", (2, DIM), F32)
    if cfg["use_qkv_b"]:
        dparam("qkv_b", (2, 128, 12), F32)
    if cfg["use_proj_b"]:
        dparam("proj_b", (2, DIM), F32)
    if cfg["use_fc1_b"]:
        dparam("fc1_b", (2, 128, 4), F32)
    if cfg["use_fc2_b"]:
        dparam("fc2_b", (2, DIM), F32)
    if cfg["use_fc_b"]:
        dparam("fc_b", (1, DIM), F32)
    if cfg["use_head_b"]:
        dparam("head_b", (1, NCLS), F32)
    if cfg["use_n1"]:
        dparam("n1_wb", (2, 2, DIM), F32)
    if cfg["use_n2"]:
        dparam("n2_wb", (2, 2, DIM), F32)
    if cfg["use_nf"]:
        dparam("nf_w", (2, DIM), F32)
        dparam("nf_b", (2, DIM), F32)
    out_p = nc.declare_dram_parameter("out", [SB, NCLS], F32, isOutput=True)
    dbg_p = None
    if debug_stage is not None:
        dbg_p = nc.declare_dram_parameter("dbgX", [128, NT, DIM], F32,
                                          isOutput=True)

    with tile.TileContext(nc) as tc:
        Kernel(tc, P, out_p, cfg, debug_stage, dbg_p).build()
    nc.finalize()   # Bacc: runs wait-splitting legalization + reg alloc
    return nc


class Kernel:
    def __init__(self, tc, P, out_p, cfg, debug_stage=None, dbg_p=None):
        self.tc, self.nc, self.P, self.out_p, self.cfg = tc, tc.nc, P, out_p, cfg
        self.debug_stage, self.dbg_p = debug_stage, dbg_p

    def dbg_dump(self, stage):
        if self.debug_stage == stage:
            self.nc.sync.dma_start(out=self.dbg_p[:], in_=self.X)

    def dbg_dump_tile(self, stage, ap):
        """Dump an arbitrary [128, N] SBUF tile into dbgX[:, 0, :N]."""
        if self.debug_stage == stage:
            n = ap.shape[-1]
            self.nc.sync.dma_start(out=self.dbg_p[:, 0, 0:n], in_=ap)

    def build(self):
        tc, nc, P = self.tc, self.nc, self.P
        with tc.tile_pool(name="const", bufs=1) as cp, \
             tc.tile_pool(name="stat", bufs=4) as stp:
            self.cp, self.stp = cp, stp
            X = cp.tile([128, NT, DIM], F32, name="X")
            self.X = X
            # X padding rows (98-127 of odd spa tiles, 31 mod 32 of spe
            # tiles) are deliberately left uninitialized: every consumer
            # either excludes them via matmul K-slices or confines their
            # garbage to the same padding lanes, and cls rows are written
            # explicitly.  Dropping the [128, 36*512] memset removes a
            # 15us Pool op that serialized the conv-stage start.
            eps_sb = cp.tile([128, 1], F32, name="eps_sb")
            nc.vector.memset(eps_sb, EPS)
            self.eps = eps_sb
            # LN stats for both uses of both blocks, filled opportunistically
            # as each chunk's last residual write lands
            self.stats = {k: cp.tile([128, NT, 2], F32, name=f"stats_{k}")
                          for k in ("A0", "M0", "A1", "M1")}

            def load(name, shape, dt, src=None, pool=None):
                t = (pool or cp).tile(list(shape), dt, name="sb_" + name)
                nc.sync.dma_start(out=t, in_=src if src is not None else P[name][:])
                return t

            self.whead = load("whead", (128, 8, NCLS), BF16)
            self.bias = {}
            for k, shp in (("qkv_b", (2, 128, 12)), ("proj_b", (2, DIM)),
                           ("fc1_b", (2, 128, 4)), ("fc2_b", (2, DIM)),
                           ("fc_b", (1, DIM)), ("head_b", (1, NCLS)),
                           ("n1_wb", (2, 2, DIM)), ("n2_wb", (2, 2, DIM)),
                           ("nf_w", (2, DIM)), ("nf_b", (2, DIM))):
                if k in P:
                    self.bias[k] = load(k, shp, F32)

            with tc.tile_pool(name="convp", bufs=1) as cvp, \
                 tc.tile_pool(name="convtmp", bufs=6) as cvt, \
                 tc.tile_pool(name="convps", bufs=1, space="PSUM") as cps:
                self.w_spa = load("w_spa", (128, 3, DIM), BF16,
                                  P["w_spa"][:].rearrange("a p m -> p a m"),
                                  pool=cvp)
                self.w_cnn = load("w_cnn", (67, 128), BF16, pool=cvp)
                self.b_cnn = load("b_cnn", (128, 1), F32, pool=cvp)
                self.w_fc = load("w_fc", (128, DIM), BF16, pool=cvp)
                self.pos_spa = load("pos_spa", (128, 2, DIM), F32, pool=cvp)
                self.pos_spe = load("pos_spe", (128, DIM), F32, pool=cvp)
                self.cls2 = load("cls2", (1, 2, DIM), F32,
                                 P["cls2"][None, :, :], pool=cvp)
                # per-engine pool accumulators (disjoint columns, but split
                # buffers keep the DVE and Act pipelines free of any shared-
                # tile bookkeeping); summed into pool_bf before the fc.
                self.pool_d = cvp.tile([128, SB * SPE_PAD], F32,
                                       name="pool_d")
                self.pool_a = cvp.tile([128, SB * SPE_PAD], F32,
                                       name="pool_a")
                nc.vector.memset(self.pool_d, 0.0)
                nc.gpsimd.memset(self.pool_a, 0.0)
                self.conv_stage(cvp, cvt, cps)

            self.dbg_dump(0)
            with tc.tile_pool(name="wblk", bufs=1) as wp, \
                 tc.tile_pool(name="blk", bufs=1) as bp, \
                 tc.tile_pool(name="grp", bufs=2) as gp, \
                 tc.tile_pool(name="attn", bufs=4) as ap, \
                 tc.tile_pool(name="small", bufs=4) as sp, \
                 tc.tile_pool(name="ps", bufs=1, space="PSUM") as ps:
                self.wp, self.bp, self.gp, self.ap, self.sp, self.ps = \
                    wp, bp, gp, ap, sp, ps
                for i in range(2):
                    if self.debug_stage is not None and self.debug_stage <= i:
                        break
                    self.block(i)
                    self.dbg_dump(i + 1)
                self.head()

    # psum helpers: one pool, explicit per-tag bufs (total <= 8 banks)
    def ps_mm(self):
        return self.ps.tile([128, DIM], F32, tag="mm", bufs=3, name="ps_mm")

    def ps_sT(self):
        return self.ps.tile([128, DIM], F32, tag="sT", bufs=3, name="ps_sT")

    def ps_o(self):
        return self.ps.tile([128, 4 * 65], F32, tag="ops", bufs=2, name="ps_o")

    # ------------------------------------------------------------ conv
    def conv_stage(self, cvp, cvt, cps):
        nc, X = self.nc, self.X
        sA0 = self.stats["A0"]
        # spa cls tokens first (host constants; conv never touches
        # partition 0 of even tiles) so spa LN stats can stream during conv.
        # spe cls must wait: the fc pos-add writes all 128 partitions.
        # On Pool: it's idle at conv start and this frees 8.6us of DVE.
        nc.gpsimd.tensor_copy(
            out=X[0:1, 0:NT_SPA:2, :],
            in_=self.cls2[0:1, 0:1, :].to_broadcast((1, SB, DIM)))
        # host-built im2cols, plain DMA loads
        im = [cvp.tile([97, SB, NPATCH], BF16, name=f"im_spa{k}")
              for k in range(3)]
        for k in range(3):
            nc.sync.dma_start(
                out=im[k],
                in_=self.P["im_spa"][k, 0:97].rearrange(
                    "k (s p) -> k s p", s=SB))
        imf = im

        def spa_stats(s):
            # one-sample delay so the X-tile DMA bounce has drained
            for c in (2 * s, 2 * s + 1):
                self.emit_stats(sA0, c)
            if c == 11:
                self.flush_stats(sA0, 0, 12)
            elif c == 23:
                self.flush_stats(sA0, 12, 24)

        for s in range(SB):
            if s >= 1:
                spa_stats(s - 1)
            for ci, (p0, p1) in enumerate(((0, 127), (127, 225))):
                m = p1 - p0
                psm = cps.tile([128, DIM], F32, tag="spaps", bufs=2,
                               name="psm_spa")
                for k in range(3):
                    kv = 97 if k == 0 else 94
                    nc.tensor.matmul(psm[:m], imf[k][:kv, s, p0:p1],
                                     self.w_spa[:kv, k, :],
                                     start=(k == 0), stop=(k == 2))
                if ci == 0:
                    # token rows 1..127 of tile 2s: partition base 1 is not
                    # engine-addressable -> bounce through DMA.
                    # Act relu -> Pool pos-add -> DMA.
                    tmp = cvt.tile([128, DIM], F32, tag="spatmp",
                                   name="tmp_spa")
                    nc.scalar.activation(out=tmp[:m], in_=psm[:m],
                                         func=AF.Relu)
                    nc.gpsimd.tensor_tensor(out=tmp[:m], in0=tmp[:m],
                                            in1=self.pos_spa[:m, ci, :],
                                            op=ALU.add)
                    nc.sync.dma_start(out=X[1:128, 2 * s, :], in_=tmp[:m])
                else:
                    # fused relu+pos-add straight into X on DVE
                    nc.vector.scalar_tensor_tensor(
                        out=X[0:98, 2 * s + 1, :], in0=psm[:m], scalar=0.0,
                        in1=self.pos_spa[:m, ci, :], op0=ALU.max, op1=ALU.add)

        spa_stats(SB - 1)

        # spectral: chunks of 15 instances (half sample); tap row = 32*kx+ky
        im2 = [cvp.tile([67, 15, NPATCH], BF16, name=f"im_spe{k}")
               for k in range(4)]
        im_spe_p = self.P["im_spe"][:].rearrange("k (i p) -> k i p", p=NPATCH)
        zeros_bf = cvp.tile([128, NPATCH], BF16, name="zeros_bf")
        nc.vector.memset(zeros_bf, 0.0)
        # Per-instance fused relu+pool on BOTH PSUM-capable engines:
        #   Act: activation(Relu, bias, accum_out)           ~560ns/inst
        #   DVE: scalar_tensor_tensor (+bias, max 0, accum)  ~359ns/inst
        # Whole (cc,g) pairs are greedy-assigned to one engine, and each
        # engine gets its own PSUM tag so the two pipelines decouple.
        DVE_NS, ACT_NS = 359, 560
        eng_load = {"d": 36000, "a": 22000}
        ninst = 0
        for cc in range(SB * 2):
            s, h2 = cc // 2, cc % 2
            t = im2[cc % 4]
            i_base = 30 * s + 15 * h2
            nc.sync.dma_start(out=t, in_=im_spe_p[:, i_base:i_base + 15, :])
            tf = t
            for g in range(8):
                i0, i1 = 2 * g, min(2 * g + 2, 15)
                n = (i1 - i0) * NPATCH
                ni = i1 - i0
                col0 = SPE_PAD * s + 1 + 15 * h2 + i0
                use_dve = (eng_load["d"] + ni * DVE_NS
                           <= eng_load["a"] + ni * ACT_NS)
                psm = cps.tile([128, 2 * NPATCH], F32,
                               tag="spepsD" if use_dve else "spepsA",
                               bufs=3, name="psm_spe")
                nc.tensor.matmul(psm[:, :n], self.w_cnn[:67, :],
                                 tf[:67, i0:i1, :], start=True, stop=True)
                for li in range(ni):
                    col = col0 + li
                    src = psm[:, li * NPATCH:(li + 1) * NPATCH]
                    if use_dve:
                        eng_load["d"] += DVE_NS
                        trash_d = cvt.tile([128, NPATCH], BF16, tag="trash_d",
                                           name="trash_d")
                        nc.vector.scalar_tensor_tensor(
                            out=trash_d, in0=src, scalar=self.b_cnn,
                            in1=zeros_bf, op0=ALU.add, op1=ALU.max,
                            accum_out=self.pool_d[:, col:col + 1])
                    else:
                        eng_load["a"] += ACT_NS
                        trash_a = cvt.tile([128, NPATCH], F32, tag="trash_a",
                                           name="trash_a")
                        nc.scalar.activation(
                            out=trash_a, in_=src, func=AF.Relu,
                            bias=self.b_cnn, scale=1.0,
                            accum_out=self.pool_a[:, col:col + 1])
                ninst += ni

        pool_bf = cvp.tile([128, SB * SPE_PAD], BF16, name="pool_bf")
        nc.vector.tensor_tensor(out=pool_bf, in0=self.pool_d,
                                in1=self.pool_a, op=ALU.add)
        self.dbg_dump_tile(10, pool_bf)
        for g in range(4):
            psm = cps.tile([128, DIM], F32, tag="spaps", bufs=2, name="psm_fc")
            nc.tensor.matmul(psm, pool_bf[:, 128 * g:128 * (g + 1)], self.w_fc,
                             start=True, stop=True)
            tmpf = cvt.tile([128, DIM], BF16, tag="fctmp", name="tmp_fc")
            nc.scalar.activation(out=tmpf, in_=psm, func=AF.Relu)
            if self.cfg["use_fc_b"]:
                nc.vector.tensor_tensor(
                    out=tmpf, in0=tmpf,
                    in1=self.bias["fc_b"][0:1, :].to_broadcast((1, DIM)),
                    op=ALU.add)
            nc.gpsimd.tensor_tensor(out=X[:, NT_SPA + g, :], in0=tmpf,
                                    in1=self.pos_spe, op=ALU.add)
        for k in range(4):
            nc.vector.tensor_copy(
                out=X[32 * k:32 * k + 1, NT_SPA:NT, :],
                in_=self.cls2[0:1, 1:2, :].to_broadcast((1, 4, DIM)))
        for g in range(4):
            self.emit_stats(sA0, NT_SPA + g)
        self.flush_stats(sA0, 24, NT)

    # ------------------------------------------------------------ layernorm
    def emit_stats(self, stats, c):
        """bn_stats+aggr for chunk c — emitted opportunistically right after
        the chunk's last residual write, so the 36-chunk serial DVE stats
        chain overlaps the previous phase instead of stalling block start."""
        nc = self.nc
        st = self.stp.tile([128, 6], F32, tag="lnst", name="st")
        nc.vector.bn_stats(out=st, in_=self.X[:, c, :])
        nc.vector.bn_aggr(out=stats[:, c, :], in_=st)

    def flush_stats(self, stats, c0, c1):
        """Finalize chunk range [c0, c1): invstd via DVE fast-rsqrt, then
        mean slot -> -mean*invstd so LN apply is x*inv + nmi (expressible
        on DVE/GpSimd and as an Act Identity activation)."""
        nc = self.nc
        self.rsqrt(stats[:, c0:c1, 1:2], c1 - c0)
        nc.vector.tensor_tensor(out=stats[:, c0:c1, 0:1],
                                in0=stats[:, c0:c1, 0:1],
                                in1=stats[:, c0:c1, 1:2], op=ALU.mult)
        nc.vector.tensor_scalar(out=stats[:, c0:c1, 0:1],
                                in0=stats[:, c0:c1, 0:1],
                                scalar1=-1.0, scalar2=None, op0=ALU.mult)

    def ln_stats(self, stats, c0, c1, grp=12):
        for c in range(c0, c1):
            self.emit_stats(stats, c)
            if c % grp == grp - 1 or c == c1 - 1:
                self.flush_stats(stats, max(c0, c - c % grp), c + 1)

    def rsqrt(self, v, n):
        """v (f32 AP [128, n]) <- 1/sqrt(v + EPS), entirely on DVE
        (fast-inverse-sqrt seed + 2 Newton steps; keeps Ln/Exp off the Act
        engine, whose table reloads thrashed against Gelu/Exp)."""
        nc = self.nc
        p = v.shape[0]
        ve = self.stp.tile([128, 16], F32, tag="rsq_ve", name="rsq_ve")
        y = self.stp.tile([128, 16], F32, tag="rsq_y", name="rsq_y")
        t = self.stp.tile([128, 16], F32, tag="rsq_t", name="rsq_t")
        vn, yn, tn = ve[:p, :n], y[:p, :n], t[:p, :n]
        nc.vector.tensor_scalar(out=vn, in0=v, scalar1=EPS, scalar2=None,
                                op0=ALU.add)
        # y0 = bitcast(0x5f3759df - (bits(ve) >> 1)); C - x == (~x) + (C+1)
        nc.vector.tensor_scalar(out=yn.bitcast(mybir.dt.int32),
                                in0=vn.bitcast(mybir.dt.int32),
                                scalar1=1, scalar2=None,
                                op0=ALU.logical_shift_right)
        nc.vector.tensor_scalar(out=yn.bitcast(mybir.dt.int32),
                                in0=yn.bitcast(mybir.dt.int32),
                                scalar1=0xFFFFFFFF, scalar2=None,
                                op0=ALU.bitwise_xor)
        nc.vector.tensor_scalar(out=yn.bitcast(mybir.dt.int32),
                                in0=yn.bitcast(mybir.dt.int32),
                                scalar1=0x5f3759df + 1, scalar2=None,
                                op0=ALU.add)
        for _ in range(2):
            nc.vector.tensor_tensor(out=tn, in0=yn, in1=yn, op=ALU.mult)
            nc.vector.tensor_tensor(out=tn, in0=tn, in1=vn, op=ALU.mult)
            nc.vector.tensor_scalar(out=tn, in0=tn, scalar1=-0.5,
                                    scalar2=1.5, op0=ALU.mult, op1=ALU.add)
            nc.vector.tensor_tensor(out=yn, in0=yn, in1=tn, op=ALU.mult)
        nc.vector.tensor_copy(out=v, in_=yn)

    def ln_apply_T(self, stats, c, dst, dst_col, affine=None):
        """LN chunk c -> DMA-transpose -> dst[:, :, dst_col:+128] (bf16).
        XBAR mapping: dst[q, e, dst_col+t] = lno[t, 128e+q] — matches the
        _pack_kT weight convention, so no repacking is needed.
        SBUF->SBUF, so 1 in 3 applies runs on the otherwise-idle GpSimd."""
        nc, X = self.nc, self.X
        lno = self.sp.tile([128, DIM], BF16, tag="lno", name="lno")
        r = c % 3
        if r == 2:
            nc.scalar.activation(out=lno, in_=X[:, c, :], func=AF.Identity,
                                 scale=stats[:, c, 1:2],
                                 bias=stats[:, c, 0:1])
        else:
            eng = nc.gpsimd if r == 1 else nc.vector
            eng.tensor_scalar(out=lno, in0=X[:, c, :],
                              scalar1=stats[:, c, 1:2],
                              scalar2=stats[:, c, 0:1],
                              op0=ALU.mult, op1=ALU.add)
        if affine is not None:
            nc.vector.tensor_tensor(out=lno, in0=lno,
                                    in1=affine[0:1, :].to_broadcast((1, DIM)),
                                    op=ALU.mult)
            nc.vector.tensor_tensor(out=lno, in0=lno,
                                    in1=affine[1:2, :].to_broadcast((1, DIM)),
                                    op=ALU.add)
        nc.sync.dma_start_transpose(
            out=dst[:, :, dst_col:dst_col + 128], in_=lno)

    # ------------------------------------------------------------ block
    def qkv_group(self, blk, xlnT_g, q_g, k_g, v_g):
        nc = self.nc
        wqk, wv = self.wblk[f"wqk{blk}"], self.wblk[f"wv{blk}"]
        qkv_b = self.bias.get("qkv_b")
        for m in range(8):
            psm = self.ps_mm()
            for kc in range(4):
                nc.tensor.matmul(psm, wqk[:, kc, 128 * m:128 * (m + 1)],
                                 xlnT_g[:, kc, :], start=(kc == 0),
                                 stop=(kc == 3))
            dst = q_g[:, m, :] if m < 4 else k_g[:, m - 4, :]
            if m < 4:
                if qkv_b is not None:
                    nc.vector.tensor_scalar(out=dst, in0=psm,
                                            scalar1=qkv_b[blk][:, m:m + 1],
                                            scalar2=float(HD) ** -0.5,
                                            op0=ALU.add, op1=ALU.mult)
                elif m % 2 == 0:
                    nc.vector.tensor_scalar(out=dst, in0=psm,
                                            scalar1=float(HD) ** -0.5,
                                            scalar2=None, op0=ALU.mult)
                else:
                    nc.scalar.activation(out=dst, in_=psm, func=AF.Copy,
                                         scale=float(HD) ** -0.5)
            else:
                if qkv_b is not None:
                    nc.vector.tensor_scalar(out=dst, in0=psm,
                                            scalar1=qkv_b[blk][:, m:m + 1],
                                            scalar2=None, op0=ALU.add)
                elif m % 2 == 0:
                    nc.vector.tensor_copy(out=dst, in_=psm)
                else:
                    nc.scalar.activation(out=dst, in_=psm, func=AF.Copy,
                                         scale=1.0)
        for t in range(4):
            psm = self.ps_mm()
            for kc in range(4):
                nc.tensor.matmul(psm, xlnT_g[:, kc, 128 * t:128 * (t + 1)],
                                 wv[:, kc, :], start=(kc == 0), stop=(kc == 3))
            if t % 2 == 0:
                nc.vector.tensor_copy(out=v_g[:, t, :, 0:64], in_=psm)
            else:
                nc.scalar.activation(out=v_g[:, t, :, 0:64], in_=psm,
                                     func=AF.Copy, scale=1.0)
        nc.vector.memset(v_g[:, :, :, 64:65], 1.0)

    def attn_sample(self, q_g, k_g, v_g, oT_g, n0, nw, mchunks, nchunks):
        """mchunks: [(tile, base, rows)]; nchunks: [(col0, rows)]."""
        nc = self.nc
        o_sb = self.ap.tile([128, 2, DIM], BF16, tag="osb", name="o_sb")
        for og in range(2):
            o_ps = [self.ps_o() for _ in nchunks]
            for hh in range(4):
                h = 4 * og + hh
                hp, hc = 64 * (h % 2), h // 2
                aTx = self.ap.tile([128, 2, 256], BF16, tag="aTx", name="aTx")
                if len(mchunks) == 2 and nw == 256:
                    # both m-chunks into one PSUM bank -> single exp op
                    # (chunk1 rows mk..127 hold stale data; excluded by the
                    # K-slice of the o-matmul, so exp of them is harmless)
                    pss = self.ps_sT()
                    for mi, (mt, mb, mk) in enumerate(mchunks):
                        nc.tensor.matmul(
                            pss[:mk, 256 * mi:256 * mi + 256],
                            k_g[hp:hp + 64, hc,
                                128 * mt + mb:128 * mt + mb + mk],
                            q_g[hp:hp + 64, hc, n0:n0 + nw],
                            start=True, stop=True)
                    nc.scalar.activation(
                        out=aTx.rearrange("p a b -> p (a b)"),
                        in_=pss, func=AF.Exp)
                else:
                    for mi, (mt, mb, mk) in enumerate(mchunks):
                        pss = self.ps_sT()
                        nc.tensor.matmul(
                            pss[:mk, :nw],
                            k_g[hp:hp + 64, hc,
                                128 * mt + mb:128 * mt + mb + mk],
                            q_g[hp:hp + 64, hc, n0:n0 + nw],
                            start=True, stop=True)
                        nc.scalar.activation(out=aTx[mb:mb + mk, mi, :nw],
                                             in_=pss[:mk, :nw], func=AF.Exp)
                for ni, (nc0, nr) in enumerate(nchunks):
                    for mi, (mt, mb, mk) in enumerate(mchunks):
                        nc.tensor.matmul(
                            o_ps[ni][:nr, 65 * hh:65 * hh + 65],
                            aTx[mb:mb + mk, mi, nc0:nc0 + nr],
                            v_g[mb:mb + mk, mt, h, :],
                            start=(mi == 0), stop=(mi == len(mchunks) - 1),
                            tile_position=(mb if mk <= 32 else 0, 0))
            for ni, (nc0, nr) in enumerate(nchunks):
                rinv = self.sp.tile([128, 8], F32, tag="rinv", name="rinv")
                nc.vector.reciprocal(out=rinv[:nr, 4 * og:4 * og + 4],
                                     in_=o_ps[ni][:nr, 64:260:65])
                # one op for all 4 heads: broadcast 1/rowsum over head dim
                src = o_ps[ni][:nr, :260].rearrange(
                    "p (h e) -> p h e", e=65)[:, :, 0:64]
                dst = o_sb[:nr, ni, 256 * og:256 * og + 256].rearrange(
                    "p (h e) -> p h e", e=64)
                nc.vector.tensor_tensor(
                    out=dst, in0=src,
                    in1=rinv[:nr, 4 * og:4 * og + 4, None].to_broadcast(
                        (nr, 4, 64)),
                    op=ALU.mult)
        # transpose o (token-major) -> oT_g feature-major columns
        for ni, (nc0, nr) in enumerate(nchunks):
            nc.sync.dma_start_transpose(
                out=oT_g[:, :, n0 + nc0:n0 + nc0 + nr],
                in_=o_sb[:nr, ni, :])

    def proj_group(self, blk, oT_g, base_tile):
        """Blanket residual: X[:, tile, :] += proj(oT). cls rows get the
        wrong (own-stream) delta here; fixed afterwards via cls_fix()."""
        nc, X = self.nc, self.X
        wproj = self.wblk[f"wproj{blk}"]
        for cc in range(4):
            psz = self.ps_mm()
            for e in range(4):
                nc.tensor.matmul(psz, oT_g[:, e, 128 * cc:128 * (cc + 1)],
                                 wproj[:, e, :], start=(e == 0), stop=(e == 3))
            if self.cfg["use_proj_b"]:
                nc.vector.tensor_tensor(
                    out=psz, in0=psz,
                    in1=self.bias["proj_b"][blk:blk + 1, :].to_broadcast((1, DIM)),
                    op=ALU.add)
            dt = base_tile + cc
            nc.vector.tensor_tensor(out=X[:, dt, :], in0=X[:, dt, :],
                                    in1=psz, op=ALU.add)

    def gather_cls(self, tx, ty):
        """DMA-gather the 32 cls rows of X into [16,512] tiles (spa, spe)."""
        nc, X = self.nc, self.X
        nc.sync.dma_start(out=tx, in_=X[0:1, 0:NT_SPA:2, :])
        for k in range(4):
            nc.sync.dma_start(out=ty[k:16:4, :],
                              in_=X[32 * k:32 * k + 1, NT_SPA:NT, :])

    def scatter_cls(self, tx, ty):
        nc, X = self.nc, self.X
        nc.sync.dma_start(out=X[0:1, 0:NT_SPA:2, :], in_=tx)
        for k in range(4):
            nc.sync.dma_start(out=X[32 * k:32 * k + 1, NT_SPA:NT, :],
                              in_=ty[k:16:4, :])

    def block(self, blk):
        nc, X, tc = self.nc, self.X, self.tc
        cfg = self.cfg
        # per-block weights
        self.wblk = {}
        for nm in ("wqk", "wv", "wproj", "wfc1", "wfc2"):
            key = nm + str(blk)
            shape = [128, 4, 2 * DIM] if nm == "wqk" else [128, 4, DIM]
            t = self.wp.tile(shape, BF16, tag=nm,
                             bufs=2 if nm == "wv" else 1, name="w_" + key)
            nc.sync.dma_start(out=t, in_=self.P[key][:])
            self.wblk[key] = t

        aff1 = self.bias["n1_wb"][blk] if cfg["use_n1"] else None
        stats1 = self.stats[f"A{blk}"]     # prefilled during previous phase
        stats2 = self.stats[f"M{blk}"]
        # snapshot cls rows before any residual update (for the cls fix)
        xc0_x = self.bp.tile([16, DIM], F32, tag="xc0x", name="xc0_x")
        xc0_y = self.bp.tile([16, DIM], F32, tag="xc0y", name="xc0_y")
        self.gather_cls(xc0_x, xc0_y)

        # LN the 16 cls tokens of each stream directly from the gathered
        # rows (per-token stats fetched from stats1), transpose once, and
        # scatter the columns into the OTHER stream's xlnT tiles.  This
        # replaces the column-swap dance and removes the dependency of the
        # spe stream on all spa LN groups (and vice versa).
        stx = self.bp.tile([16, 2], F32, tag="stx", name="stx")
        nc.sync.dma_start(out=stx, in_=stats1[0:1, 0:NT_SPA:2, :])
        sty = self.bp.tile([16, 2], F32, tag="sty", name="sty")
        for k in range(4):
            nc.sync.dma_start(out=sty[k:16:4, :],
                              in_=stats1[32 * k:32 * k + 1, NT_SPA:NT, :])
        clsln = []
        for nm, xc, st in (("x", xc0_x, stx), ("y", xc0_y, sty)):
            cl = self.bp.tile([16, DIM], BF16, tag=f"cls{nm}", name=f"cls{nm}")
            nc.vector.tensor_scalar(out=cl, in0=xc, scalar1=st[:, 1:2],
                                    scalar2=st[:, 0:1],
                                    op0=ALU.mult, op1=ALU.add)
            if aff1 is not None:
                nc.vector.tensor_tensor(out=cl, in0=cl,
                                        in1=aff1[0:1, :].to_broadcast((1, DIM)),
                                        op=ALU.mult)
                nc.vector.tensor_tensor(out=cl, in0=cl,
                                        in1=aff1[1:2, :].to_broadcast((1, DIM)),
                                        op=ALU.add)
            clT = self.bp.tile([128, 4, 16], BF16, tag=f"cls{nm}T",
                               name=f"cls{nm}T")
            nc.sync.dma_start_transpose(out=clT, in_=cl)
            clsln.append(clT)
        clsxT, clsyT = clsln

        # Pre-double the cls rows: the blanket residual with SWAPPED oT
        # columns then produces exactly 2*x_cls + proj(other-stream cls out),
        # eliminating the old gather/fix/scatter sync between attention and
        # MLP.  (xlnT never reads X cls rows — their columns are overwritten
        # by the cls scatters — and stats1/xc0 were captured above.)
        nc.vector.tensor_scalar(out=X[0:1, 0:NT_SPA:2, :],
                                in0=X[0:1, 0:NT_SPA:2, :], scalar1=2.0,
                                scalar2=None, op0=ALU.mult)
        for k in range(4):
            nc.vector.tensor_scalar(out=X[32 * k:32 * k + 1, NT_SPA:NT, :],
                                    in0=X[32 * k:32 * k + 1, NT_SPA:NT, :],
                                    scalar1=2.0, scalar2=None, op0=ALU.mult)

        xlnT_spe = self.bp.tile([128, 4, DIM], BF16, tag="xlnTspe",
                                name="xlnT_spe")
        for lc in range(4):
            self.ln_apply_T(stats1, NT_SPA + lc, xlnT_spe, 128 * lc, aff1)
        nc.gpsimd.tensor_copy(out=xlnT_spe[:, :, 0:SB * 32:32], in_=clsxT)

        q_s = self.gp.tile([128, 4, DIM], BF16, tag="qs", bufs=1, name="q_s")
        k_s = self.gp.tile([128, 4, DIM], BF16, tag="ks", bufs=1, name="k_s")
        v_s = self.gp.tile([128, 4, NH, 65], BF16, tag="vs", bufs=1,
                           name="v_s")
        oT_s = self.gp.tile([128, 4, DIM], BF16, tag="oTs", bufs=1,
                            name="oT_s")
        # spe attn samples 2g, 2g+1 run inside spa group g: the cls-column
        # swap before proj g needs them
        spe_sched = {g: (2 * g, 2 * g + 2) for g in range(8)}

        spa_nchunks = [(0, 128), (128, 128)]
        for g in range(8):
            xlnT_g = self.gp.tile([128, 4, DIM], BF16, tag="xlnT", name="xlnT_g")
            for lc in range(4):
                self.ln_apply_T(stats1, 4 * g + lc, xlnT_g, 128 * lc, aff1)
            nc.gpsimd.tensor_copy(out=xlnT_g[:, :, 0:DIM:256],
                                  in_=clsyT[:, :, 2 * g:2 * g + 2])
            q_g = self.gp.tile([128, 4, DIM], BF16, tag="qg", name="q_g")
            k_g = self.gp.tile([128, 4, DIM], BF16, tag="kg", name="k_g")
            v_g = self.gp.tile([128, 4, NH, 65], BF16, tag="vg", name="v_g")
            self.qkv_group(blk, xlnT_g, q_g, k_g, v_g)
            if g == 0:
                self.qkv_group(blk, xlnT_spe, q_s, k_s, v_s)
            for s in range(*spe_sched.get(g, (0, 0))):
                self.attn_sample(q_s, k_s, v_s, oT_s, 32 * s, 32,
                                 [(s // 4, 32 * (s % 4), 31)], [(0, 32)])
            oT_g = self.gp.tile([128, 4, DIM], BF16, tag="oTg", name="oT_g")
            for j in range(2):
                self.attn_sample(q_g, k_g, v_g, oT_g, 256 * j, 256,
                                 [(2 * j, 0, 128), (2 * j + 1, 0, 98)],
                                 spa_nchunks)
            # swap the two streams' cls attention-output columns pre-proj
            for j in range(2):
                s = 2 * g + j
                ctm = self.sp.tile([128, 4, 1], BF16, tag="ctm", name="ctm")
                nc.gpsimd.tensor_copy(out=ctm,
                                      in_=oT_s[:, :, 32 * s:32 * s + 1])
                nc.gpsimd.tensor_copy(out=oT_s[:, :, 32 * s:32 * s + 1],
                                      in_=oT_g[:, :, 256 * j:256 * j + 1])
                nc.gpsimd.tensor_copy(out=oT_g[:, :, 256 * j:256 * j + 1],
                                      in_=ctm)
            self.proj_group(blk, oT_g, 4 * g)
            for c in range(4 * g, 4 * g + 4):
                self.emit_stats(stats2, c)
            self.flush_stats(stats2, 4 * g, 4 * g + 4)
            # one-group delay: group g-1's stats flushed last iteration, so
            # this MLP never stalls an in-order queue mid-attention
            if g >= 2 and g % 2 == 0:
                self.mlp_group(blk, stats2, g - 2)
                self.mlp_group(blk, stats2, g - 1)

        # spe stream tail: proj with fully-swapped oT_s, then trailing MLPs
        self.proj_group(blk, oT_s, NT_SPA)
        for c in range(NT_SPA, NT):
            self.emit_stats(stats2, c)
        self.flush_stats(stats2, NT_SPA, NT)
        self.mlp_group(blk, stats2, 6)
        self.mlp_group(blk, stats2, 7)
        self.mlp_group(blk, stats2, 8)

    def mlp_group(self, blk, stats2, r):
        nc, X, cfg = self.nc, self.X, self.cfg
        wfc1, wfc2 = self.wblk[f"wfc1{blk}"], self.wblk[f"wfc2{blk}"]
        aff2 = self.bias["n2_wb"][blk] if cfg["use_n2"] else None
        fc1_b = self.bias.get("fc1_b")
        x2T = self.gp.tile([128, 4, DIM], BF16, tag="x2T", name="x2T")
        for lc in range(4):
            self.ln_apply_T(stats2, 4 * r + lc, x2T, 128 * lc, aff2)
        h1T = self.gp.tile([128, 4, DIM], BF16, tag="h1T", name="h1T")
        for m in range(4):
            psm = self.ps_mm()
            for kc in range(4):
                nc.tensor.matmul(psm, wfc1[:, kc, 128 * m:128 * (m + 1)],
                                 x2T[:, kc, :], start=(kc == 0),
                                 stop=(kc == 3))
            b1 = fc1_b[blk][:, m:m + 1] if fc1_b is not None else 0.0
            nc.scalar.activation(out=h1T[:, m, :], in_=psm, func=AF.Gelu,
                                 bias=b1, scale=1.0)
        for cc in range(4):
            psm = self.ps_mm()
            for kc in range(4):
                nc.tensor.matmul(psm, h1T[:, kc, 128 * cc:128 * (cc + 1)],
                                 wfc2[:, kc, :], start=(kc == 0),
                                 stop=(kc == 3))
            if cfg["use_fc2_b"]:
                nc.vector.tensor_tensor(
                    out=psm, in0=psm,
                    in1=self.bias["fc2_b"][blk:blk + 1, :].to_broadcast(
                        (1, DIM)), op=ALU.add)
            dt = 4 * r + cc
            nc.vector.tensor_tensor(out=X[:, dt, :], in0=X[:, dt, :],
                                    in1=psm, op=ALU.add)
        if blk == 0:
            sA1 = self.stats["A1"]
            for c in range(4 * r, 4 * r + 4):
                self.emit_stats(sA1, c)
            if r % 3 == 2:
                self.flush_stats(sA1, 12 * (r // 3), 12 * (r // 3) + 12)

    # ------------------------------------------------------------ head
    def head(self):
        nc, X, cfg = self.nc, self.X, self.cfg
        xcf = [self.bp.tile([16, DIM], F32, tag=f"xcf{h}", name=f"xcf{h}")
               for h in range(2)]
        self.gather_cls(xcf[0], xcf[1])
        clsn = [self.bp.tile([16, DIM], BF16, tag=f"clsn{h}", name=f"clsn{h}")
                for h in range(2)]
        for half in range(2):
            st = self.sp.tile([16, 6], F32, tag="lnstf", name="stf")
            mv = self.sp.tile([16, 2], F32, tag="fmv", name="mvf")
            nc.vector.bn_stats(out=st, in_=xcf[half])
            nc.vector.bn_aggr(out=mv, in_=st)
            self.rsqrt(mv[:, 1:2], 1)
            nc.vector.tensor_scalar(out=clsn[half], in0=xcf[half],
                                    scalar1=mv[:, 0:1], scalar2=mv[:, 1:2],
                                    op0=ALU.subtract, op1=ALU.mult)
            if cfg["use_nf"]:
                nc.vector.tensor_tensor(
                    out=clsn[half], in0=clsn[half],
                    in1=self.bias["nf_w"][half:half + 1, :].to_broadcast((1, DIM)),
                    op=ALU.mult)
                nc.vector.tensor_tensor(
                    out=clsn[half], in0=clsn[half],
                    in1=self.bias["nf_b"][half:half + 1, :].to_broadcast((1, DIM)),
                    op=ALU.add)
        clsT = self.bp.tile([128, 8, NCLS], BF16, tag="clsT", name="clsT")
        for half in range(2):
            nc.sync.dma_start_transpose(
                out=clsT[:, 4 * half:4 * half + 4, :], in_=clsn[half])
        psh = self.ps.tile([16, NCLS], F32, tag="mm", bufs=3, name="psh")
        for dc in range(8):
            nc.tensor.matmul(psh, clsT[:, dc, :], self.whead[:, dc, :],
                             start=(dc == 0), stop=(dc == 7))
        out_sb = self.bp.tile([16, NCLS], F32, tag="out_sb", name="out_sb")
        if cfg["use_head_b"]:
            nc.vector.tensor_tensor(
                out=out_sb, in0=psh,
                in1=self.bias["head_b"][0:1, :].to_broadcast((1, NCLS)),
                op=ALU.add)
        else:
            nc.vector.tensor_copy(out=out_sb, in_=psh)
        nc.sync.dma_start(out=self.out_p[:], in_=out_sb)


# ---------------------------------------------------------------- entry
def kernel(**inputs):
    w = prepare_weights(inputs)
    nc = build_program(w)
    in_maps = make_in_maps(inputs, w)
    res = run_bass_kernel_spmd(nc, in_maps, list(range(NCORES)))
    out = np.concatenate([np.asarray(r["out"], np.float32)
                          for r in res.results], axis=0)
    return out


def run_traced(inputs):
    """For test.py: returns (out, BassKernelResults with exec_time_ns)."""
    w = prepare_weights(inputs)
    nc = build_program(w)
    in_maps = make_in_maps(inputs, w)
    res = run_bass_kernel_spmd(nc, in_maps, list(range(NCORES)), trace=True)
    out = np.concatenate([np.asarray(r["out"], np.float32)
                          for r in res.results], axis=0)
    return out, res



# revision 17
# speedup vs baseline: 1.0121x; 1.0121x over previous
"""Trainium2 Bass kernel for nn_CASST (dense transformer, CTMF blocks).

Self-contained: builds the Bass program from the concrete numpy inputs,
shards batch B=128 across 8 NeuronCores (16 samples each), runs SPMD,
gathers the full [128, 16] output.

Per-core layout:
  X [128, 36, 512] f32 token-major residual: tiles 0..31 = spatial stream
  (16 samples x 256 padded rows, 226 valid: cls at row 0, patches 1..225),
  tiles 32..35 = spectral stream (16 samples x 32 padded rows, 31 valid).
  Matmul operands bf16; PSUM accumulation f32.
  Attention: scores computed transposed (keys on partitions) so softmax
  needs no transpose of the attention matrix; the row-sum comes free from
  an extra all-ones column appended to V, and the normalization is fused
  into the PSUM->SBUF copy of the per-head output.
"""
import sys

sys.path.insert(0, "/opt/trn_rl_repo")

import numpy as np
import ml_dtypes

import concourse.bass as bass
import concourse.tile as tile
from concourse import bacc
from concourse import mybir
from concourse.masks import make_identity
from concourse.bass_utils import run_bass_kernel_spmd

F32 = mybir.dt.float32
BF16 = mybir.dt.bfloat16
AF = mybir.ActivationFunctionType
ALU = mybir.AluOpType

B, BANDS, HW, DIM, NH, NCLS = 128, 30, 15, 512, 8, 16
NCORES = 8
SB = B // NCORES          # 16 samples per core
NPATCH = HW * HW          # 225
SPA_PAD, SPE_PAD = 256, 32
NT_SPA = SB * SPA_PAD // 128   # 32
NT_SPE = SB * SPE_PAD // 128   # 4
NT = NT_SPA + NT_SPE           # 36
EPS = 1e-5
HD = DIM // NH            # 64


def _bf(x):
    return np.asarray(x, dtype=np.float32).astype(ml_dtypes.bfloat16)


def _pack_kT(w_T, mdim):
    """[512, M] (contraction rows) -> [128, 4, M] (partition, k-chunk, M)."""
    k = w_T.shape[0]
    return np.ascontiguousarray(w_T.reshape(k // 128, 128, mdim).transpose(1, 0, 2))


def _ident(a, b):
    return bool(np.all(np.asarray(a) == 1) and np.all(np.asarray(b) == 0))


def prepare_weights(inp):
    w = {}
    # spatial conv + BN fold
    s_h = inp["bn_h_g"] / np.sqrt(inp["bn_h_v"] + EPS)
    b_h = inp["conv_h_b"] * s_h + inp["bn_h_b"] - inp["bn_h_m"] * s_h
    w_h = np.asarray(inp["conv_h_w"]) * np.asarray(s_h)[:, None, None, None]
    # K-tile ky: row 32*kx + b holds tap (ky,kx) band b; tile0 row 96 = bias
    w_spa = np.zeros((3, 128, DIM), np.float32)
    for ky in range(3):
        for kx in range(3):
            w_spa[ky, 32 * kx:32 * kx + 30, :] = np.asarray(w_h)[:, :, ky, kx].T
    w_spa[0, 96, :] = np.asarray(b_h)
    w["w_spa"] = _bf(w_spa)

    # spectral conv + BN fold
    s_c = inp["cnn_bn_g"] / np.sqrt(inp["cnn_bn_v"] + EPS)
    b_c = inp["cnn_conv_b"] * s_c + inp["cnn_bn_b"] - inp["cnn_bn_m"] * s_c
    w_c = np.asarray(inp["cnn_conv_w"]) * np.asarray(s_c)[:, None, None, None]
    # row 32*kx + ky holds tap (ky,kx) so border memsets stay 32-aligned
    w_cnn = np.zeros((67, 128), np.float32)
    for ky in range(3):
        for kx in range(3):
            w_cnn[32 * kx + ky, :] = w_c[:, 0, ky, kx]
    w["w_cnn"] = _bf(w_cnn)
    w["b_cnn"] = np.asarray(b_c, np.float32).reshape(128, 1)
    w["w_fc"] = _bf(np.asarray(inp["cnn_fc_w"]).T / NPATCH)   # mean folded
    w["fc_b"] = np.asarray(inp["cnn_fc_b"], np.float32)

    for i in range(2):
        qkv = np.asarray(inp["blk_qkv_w"][i])
        wq, wk, wv = qkv[:DIM], qkv[DIM:2 * DIM], qkv[2 * DIM:]
        w[f"wqk{i}"] = _bf(_pack_kT(np.concatenate([wq.T, wk.T], 1), 2 * DIM))
        w[f"wv{i}"] = _bf(_pack_kT(wv.T, DIM))
        w[f"wproj{i}"] = _bf(_pack_kT(np.asarray(inp["blk_proj_w"][i]).T, DIM))
        w[f"wfc1{i}"] = _bf(_pack_kT(np.asarray(inp["blk_fc1_w"][i]).T, DIM))
        w[f"wfc2{i}"] = _bf(_pack_kT(np.asarray(inp["blk_fc2_w"][i]).T, DIM))
    w["whead"] = _bf(_pack_kT(np.asarray(inp["head_w"]).T, NCLS))

    for k in ("blk_qkv_b", "blk_proj_b", "blk_fc1_b", "blk_fc2_b", "head_b",
              "blk_n1_w", "blk_n1_b", "blk_n2_w", "blk_n2_b",
              "norm1_w", "norm1_b", "norm2_w", "norm2_b"):
        w[k] = np.asarray(inp[k], np.float32)

    # pos_spa[p, c] = positional embedding for the token that conv-psum row p
    # of chunk c produces: chunk0 row p -> token p+1, chunk1 row p -> token 128+p
    pos_shift = np.zeros((128, 2, DIM), np.float32)
    spa_pos = np.asarray(inp["spa_pos"])[0]           # [226, 512]
    pos_shift[0:127, 0] = spa_pos[1:128]
    pos_shift[0:98, 1] = spa_pos[128:226]
    w["pos_spa"] = pos_shift
    pos_spe = np.zeros((SPE_PAD, DIM), np.float32)
    pos_spe[1:1 + BANDS] = np.asarray(inp["spe_pos"])[0, 1:1 + BANDS]
    w["pos_spe"] = np.ascontiguousarray(np.tile(pos_spe, (4, 1)))
    cls2 = np.zeros((2, DIM), np.float32)
    cls2[0] = np.asarray(inp["spa_cls"])[0, 0] + np.asarray(inp["spa_pos"])[0, 0]
    cls2[1] = np.asarray(inp["spe_cls"])[0, 0] + np.asarray(inp["spe_pos"])[0, 0]
    w["cls2"] = cls2

    w["cfg"] = dict(
        use_qkv_b=bool(np.any(w["blk_qkv_b"] != 0)),
        use_proj_b=bool(np.any(w["blk_proj_b"] != 0)),
        use_fc1_b=bool(np.any(w["blk_fc1_b"] != 0)),
        use_fc2_b=bool(np.any(w["blk_fc2_b"] != 0)),
        use_fc_b=bool(np.any(w["fc_b"] != 0)),
        use_head_b=bool(np.any(w["head_b"] != 0)),
        use_n1=not all(_ident(w["blk_n1_w"][i], w["blk_n1_b"][i]) for i in range(2)),
        use_n2=not all(_ident(w["blk_n2_w"][i], w["blk_n2_b"][i]) for i in range(2)),
        use_nf=not (_ident(w["norm1_w"], w["norm1_b"])
                    and _ident(w["norm2_w"], w["norm2_b"])),
    )
    return w


def _im2cols(xc):
    """Host im2col for one core's x shard [SB, 30, 15, 15] (f32)."""
    xp = np.pad(xc, ((0, 0), (0, 0), (1, 1), (1, 1)))
    im_spa = np.zeros((3, 128, SB * NPATCH), np.float32)
    im_spe = np.zeros((67, SB * BANDS * NPATCH), np.float32)
    for ky in range(3):
        for kx in range(3):
            win = xp[:, :, ky:ky + HW, kx:kx + HW]          # [SB,30,15,15]
            im_spa[ky, 32 * kx:32 * kx + 30, :] = (
                win.transpose(1, 0, 2, 3).reshape(BANDS, -1))
            im_spe[32 * kx + ky, :] = win.reshape(-1)
    im_spa[0, 96, :] = 1.0
    return _bf(im_spa), _bf(im_spe)


def make_in_maps(inputs, w):
    x = np.asarray(inputs["x"], np.float32)[:, 0]   # [128, 30, 15, 15]
    cfg = w["cfg"]
    base = {k: w[k] for k in
            ("w_spa", "w_cnn", "b_cnn", "w_fc", "whead", "pos_spa",
             "pos_spe", "cls2")}
    for i in range(2):
        for nm in ("wqk", "wv", "wproj", "wfc1", "wfc2"):
            base[nm + str(i)] = w[nm + str(i)]
    if cfg["use_qkv_b"]:
        base["qkv_b"] = np.ascontiguousarray(
            w["blk_qkv_b"].reshape(2, 12, 128).transpose(0, 2, 1))
    if cfg["use_proj_b"]:
        base["proj_b"] = w["blk_proj_b"]
    if cfg["use_fc1_b"]:
        base["fc1_b"] = np.ascontiguousarray(
            w["blk_fc1_b"].reshape(2, 4, 128).transpose(0, 2, 1))
    if cfg["use_fc2_b"]:
        base["fc2_b"] = w["blk_fc2_b"]
    if cfg["use_fc_b"]:
        base["fc_b"] = w["fc_b"].reshape(1, DIM)
    if cfg["use_head_b"]:
        base["head_b"] = w["head_b"].reshape(1, NCLS)
    if cfg["use_n1"]:
        base["n1_wb"] = np.ascontiguousarray(
            np.stack([w["blk_n1_w"], w["blk_n1_b"]], axis=1))
    if cfg["use_n2"]:
        base["n2_wb"] = np.ascontiguousarray(
            np.stack([w["blk_n2_w"], w["blk_n2_b"]], axis=1))
    if cfg["use_nf"]:
        base["nf_w"] = np.stack([w["norm1_w"], w["norm2_w"]])
        base["nf_b"] = np.stack([w["norm1_b"], w["norm2_b"]])
    maps = []
    for c in range(NCORES):
        m = dict(base)
        m["im_spa"], m["im_spe"] = _im2cols(x[c * SB:(c + 1) * SB])
        maps.append(m)
    return maps


# =====================================================================
def build_program(w, debug_stage=None):
    cfg = w["cfg"]
    nc = bacc.Bacc(None)
    P = {}

    def dparam(name, shape, dt):
        P[name] = nc.declare_dram_parameter(name, list(shape), dt, isOutput=False)

    dparam("im_spa", (3, 128, SB * NPATCH), BF16)
    dparam("im_spe", (67, SB * BANDS * NPATCH), BF16)
    dparam("w_spa", (3, 128, DIM), BF16)
    dparam("w_cnn", (67, 128), BF16)
    dparam("b_cnn", (128, 1), F32)
    dparam("w_fc", (128, DIM), BF16)
    for i in range(2):
        dparam(f"wqk{i}", (128, 4, 2 * DIM), BF16)
        for nm in ("wv", "wproj", "wfc1", "wfc2"):
            dparam(f"{nm}{i}", (128, 4, DIM), BF16)
    dparam("whead", (128, 8, NCLS), BF16)
    dparam("pos_spa", (128, 2, DIM), F32)
    dparam("pos_spe", (128, DIM), F32)
    dparam("cls2", (2, DIM), F32)
    if cfg["use_qkv_b"]:
        dparam("qkv_b", (2, 128, 12), F32)
    if cfg["use_proj_b"]:
        dparam("proj_b", (2, DIM), F32)
    if cfg["use_fc1_b"]:
        dparam("fc1_b", (2, 128, 4), F32)
    if cfg["use_fc2_b"]:
        dparam("fc2_b", (2, DIM), F32)
    if cfg["use_fc_b"]:
        dparam("fc_b", (1, DIM), F32)
    if cfg["use_head_b"]:
        dparam("head_b", (1, NCLS), F32)
    if cfg["use_n1"]:
        dparam("n1_wb", (2, 2, DIM), F32)
    if cfg["use_n2"]:
        dparam("n2_wb", (2, 2, DIM), F32)
    if cfg["use_nf"]:
        dparam("nf_w", (2, DIM), F32)
        dparam("nf_b", (2, DIM), F32)
    out_p = nc.declare_dram_parameter("out", [SB, NCLS], F32, isOutput=True)
    dbg_p = None
    if debug_stage is not None:
        dbg_p = nc.declare_dram_parameter("dbgX", [128, NT, DIM], F32,
                                          isOutput=True)

    with tile.TileContext(nc) as tc:
        Kernel(tc, P, out_p, cfg, debug_stage, dbg_p).build()
    nc.finalize()   # Bacc: runs wait-splitting legalization + reg alloc
    return nc


class Kernel:
    def __init__(self, tc, P, out_p, cfg, debug_stage=None, dbg_p=None):
        self.tc, self.nc, self.P, self.out_p, self.cfg = tc, tc.nc, P, out_p, cfg
        self.debug_stage, self.dbg_p = debug_stage, dbg_p

    def dbg_dump(self, stage):
        if self.debug_stage == stage:
            self.nc.sync.dma_start(out=self.dbg_p[:], in_=self.X)

    def dbg_dump_tile(self, stage, ap):
        """Dump an arbitrary [128, N] SBUF tile into dbgX[:, 0, :N]."""
        if self.debug_stage == stage:
            n = ap.shape[-1]
            self.nc.sync.dma_start(out=self.dbg_p[:, 0, 0:n], in_=ap)

    def build(self):
        tc, nc, P = self.tc, self.nc, self.P
        with tc.tile_pool(name="const", bufs=1) as cp, \
             tc.tile_pool(name="stat", bufs=4) as stp:
            self.cp, self.stp = cp, stp
            X = cp.tile([128, NT, DIM], F32, name="X")
            self.X = X
            # X padding rows (98-127 of odd spa tiles, 31 mod 32 of spe
            # tiles) are deliberately left uninitialized: every consumer
            # either excludes them via matmul K-slices or confines their
            # garbage to the same padding lanes, and cls rows are written
            # explicitly.  Dropping the [128, 36*512] memset removes a
            # 15us Pool op that serialized the conv-stage start.
            eps_sb = cp.tile([128, 1], F32, name="eps_sb")
            nc.vector.memset(eps_sb, EPS)
            self.eps = eps_sb
            # LN stats for both uses of both blocks, filled opportunistically
            # as each chunk's last residual write lands
            self.stats = {k: cp.tile([128, NT, 2], F32, name=f"stats_{k}")
                          for k in ("A0", "M0", "A1", "M1")}

            def load(name, shape, dt, src=None, pool=None):
                t = (pool or cp).tile(list(shape), dt, name="sb_" + name)
                nc.sync.dma_start(out=t, in_=src if src is not None else P[name][:])
                return t

            self.whead = load("whead", (128, 8, NCLS), BF16)
            self.bias = {}
            for k, shp in (("qkv_b", (2, 128, 12)), ("proj_b", (2, DIM)),
                           ("fc1_b", (2, 128, 4)), ("fc2_b", (2, DIM)),
                           ("fc_b", (1, DIM)), ("head_b", (1, NCLS)),
                           ("n1_wb", (2, 2, DIM)), ("n2_wb", (2, 2, DIM)),
                           ("nf_w", (2, DIM)), ("nf_b", (2, DIM))):
                if k in P:
                    self.bias[k] = load(k, shp, F32)

            with tc.tile_pool(name="convp", bufs=1) as cvp, \
                 tc.tile_pool(name="convtmp", bufs=6) as cvt, \
                 tc.tile_pool(name="convps", bufs=1, space="PSUM") as cps:
                self.w_spa = load("w_spa", (128, 3, DIM), BF16,
                                  P["w_spa"][:].rearrange("a p m -> p a m"),
                                  pool=cvp)
                self.w_cnn = load("w_cnn", (67, 128), BF16, pool=cvp)
                self.b_cnn = load("b_cnn", (128, 1), F32, pool=cvp)
                self.w_fc = load("w_fc", (128, DIM), BF16, pool=cvp)
                self.pos_spa = load("pos_spa", (128, 2, DIM), F32, pool=cvp)
                self.pos_spe = load("pos_spe", (128, DIM), F32, pool=cvp)
                self.cls2 = load("cls2", (1, 2, DIM), F32,
                                 P["cls2"][None, :, :], pool=cvp)
                # per-engine pool accumulators: each pooled column is written
                # by exactly one op into exactly one buffer (rest stay 0), so
                # no pipeline ever shares a destination tile.  DVE and Pool
                # get ping-pong pairs: consecutive accum writes to adjacent
                # columns of one tile cost a ~160ns WAW stall otherwise.
                self.pool_d = [cvp.tile([128, SB * SPE_PAD], F32,
                                        name=f"pool_d{i}") for i in range(2)]
                self.pool_p = [cvp.tile([128, SB * SPE_PAD], F32,
                                        name=f"pool_p{i}") for i in range(2)]
                self.pool_a = cvp.tile([128, SB * SPE_PAD], F32,
                                       name="pool_a")
                for t in self.pool_d:
                    nc.vector.memset(t, 0.0)
                for t in self.pool_p:
                    nc.gpsimd.memset(t, 0.0)
                nc.gpsimd.memset(self.pool_a, 0.0)
                self.conv_stage(cvp, cvt, cps)

            self.dbg_dump(0)
            with tc.tile_pool(name="wblk", bufs=1) as wp, \
                 tc.tile_pool(name="blk", bufs=1) as bp, \
                 tc.tile_pool(name="grp", bufs=2) as gp, \
                 tc.tile_pool(name="attn", bufs=4) as ap, \
                 tc.tile_pool(name="small", bufs=4) as sp, \
                 tc.tile_pool(name="ps", bufs=1, space="PSUM") as ps:
                self.wp, self.bp, self.gp, self.ap, self.sp, self.ps = \
                    wp, bp, gp, ap, sp, ps
                for i in range(2):
                    if self.debug_stage is not None and self.debug_stage <= i:
                        break
                    self.block(i)
                    self.dbg_dump(i + 1)
                self.head()

    # psum helpers: one pool, explicit per-tag bufs (total <= 8 banks)
    def ps_mm(self):
        return self.ps.tile([128, DIM], F32, tag="mm", bufs=3, name="ps_mm")

    def ps_sT(self):
        return self.ps.tile([128, DIM], F32, tag="sT", bufs=3, name="ps_sT")

    def ps_o(self):
        return self.ps.tile([128, 4 * 65], F32, tag="ops", bufs=2, name="ps_o")

    # ------------------------------------------------------------ conv
    def conv_stage(self, cvp, cvt, cps):
        nc, X = self.nc, self.X
        sA0 = self.stats["A0"]
        # spa cls tokens first (host constants; conv never touches
        # partition 0 of even tiles) so spa LN stats can stream during conv.
        # spe cls must wait: the fc pos-add writes all 128 partitions.
        # On Pool: it's idle at conv start and this frees 8.6us of DVE.
        nc.gpsimd.tensor_copy(
            out=X[0:1, 0:NT_SPA:2, :],
            in_=self.cls2[0:1, 0:1, :].to_broadcast((1, SB, DIM)))
        # host-built im2cols, plain DMA loads
        im = [cvp.tile([97, SB, NPATCH], BF16, name=f"im_spa{k}")
              for k in range(3)]
        for k in range(3):
            nc.sync.dma_start(
                out=im[k],
                in_=self.P["im_spa"][k, 0:97].rearrange(
                    "k (s p) -> k s p", s=SB))
        imf = im

        def spa_stats(s):
            # one-sample delay so the X-tile DMA bounce has drained
            for c in (2 * s, 2 * s + 1):
                self.emit_stats(sA0, c)
            if c == 11:
                self.flush_stats(sA0, 0, 12)
            elif c == 23:
                self.flush_stats(sA0, 12, 24)

        for s in range(SB):
            if s >= 1:
                spa_stats(s - 1)
            for ci, (p0, p1) in enumerate(((0, 127), (127, 225))):
                m = p1 - p0
                psm = cps.tile([128, DIM], F32, tag="spaps", bufs=2,
                               name="psm_spa")
                for k in range(3):
                    kv = 97 if k == 0 else 94
                    nc.tensor.matmul(psm[:m], imf[k][:kv, s, p0:p1],
                                     self.w_spa[:kv, k, :],
                                     start=(k == 0), stop=(k == 2))
                if ci == 0:
                    # token rows 1..127 of tile 2s: partition base 1 is not
                    # engine-addressable -> bounce through DMA.
                    # Act relu -> Pool pos-add -> DMA.
                    tmp = cvt.tile([128, DIM], F32, tag="spatmp",
                                   name="tmp_spa")
                    nc.scalar.activation(out=tmp[:m], in_=psm[:m],
                                         func=AF.Relu)
                    nc.gpsimd.tensor_tensor(out=tmp[:m], in0=tmp[:m],
                                            in1=self.pos_spa[:m, ci, :],
                                            op=ALU.add)
                    nc.sync.dma_start(out=X[1:128, 2 * s, :], in_=tmp[:m])
                else:
                    # fused relu+pos-add straight into X on DVE
                    nc.vector.scalar_tensor_tensor(
                        out=X[0:98, 2 * s + 1, :], in0=psm[:m], scalar=0.0,
                        in1=self.pos_spa[:m, ci, :], op0=ALU.max, op1=ALU.add)

        spa_stats(SB - 1)

        # spectral: chunks of 15 instances (half sample); tap row = 32*kx+ky
        im2 = [cvp.tile([67, 15, NPATCH], BF16, name=f"im_spe{k}")
               for k in range(4)]
        im_spe_p = self.P["im_spe"][:].rearrange("k (i p) -> k i p", p=NPATCH)
        zeros_bf = cvp.tile([128, NPATCH], BF16, name="zeros_bf")
        nc.vector.memset(zeros_bf, 0.0)
        # relu+pool per (cc,g) pair via one of four job types, greedy-
        # balanced over the three non-PE engines (modeled per-pair ns):
        #   df: DVE fused stt+accum per instance          DVE 718
        #   af: Act fused relu+accum per instance         Act 1188 (aux read)
        #   pa: Act relu pair -> Pool stt+accum per inst  Act 560, Pool 814
        #   pd: DVE relu pair -> Pool stt+accum per inst  DVE 594, Pool 814
        eng_load = {"d": 36000, "a": 22000, "p": 30000}
        JOBS = {"df": {"d": 718}, "af": {"a": 1188},
                "pa": {"a": 560, "p": 814}, "pd": {"d": 594, "p": 814}}
        ninst = 0
        for cc in range(SB * 2):
            s, h2 = cc // 2, cc % 2
            t = im2[cc % 4]
            i_base = 30 * s + 15 * h2
            nc.sync.dma_start(out=t, in_=im_spe_p[:, i_base:i_base + 15, :])
            tf = t
            for g in range(8):
                i0, i1 = 2 * g, min(2 * g + 2, 15)
                n = (i1 - i0) * NPATCH
                ni = i1 - i0
                col0 = SPE_PAD * s + 1 + 15 * h2 + i0
                best, best_key = None, None
                for jt, cost in JOBS.items():
                    trial = dict(eng_load)
                    for e, c in cost.items():
                        trial[e] += c * ni / 2
                    key = (max(trial.values()), sum(trial.values()))
                    if best is None or key < best:
                        best, best_key = key, jt
                jt = best_key
                for e, c in JOBS[jt].items():
                    eng_load[e] += c * ni / 2
                psm = cps.tile([128, 2 * NPATCH], F32,
                               tag="spepsD" if jt in ("df", "pd") else "spepsA",
                               bufs=3, name="psm_spe")
                nc.tensor.matmul(psm[:, :n], self.w_cnn[:67, :],
                                 tf[:67, i0:i1, :], start=True, stop=True)
                if jt in ("pa", "pd"):
                    relu_p = cvt.tile([128, 2 * NPATCH], BF16,
                                      tag="relu_" + jt, name="relu_p")
                    if jt == "pa":
                        nc.scalar.activation(out=relu_p[:, :n], in_=psm[:, :n],
                                             func=AF.Relu, bias=self.b_cnn,
                                             scale=1.0)
                    else:
                        nc.vector.tensor_scalar(
                            out=relu_p[:, :n], in0=psm[:, :n],
                            scalar1=self.b_cnn, scalar2=0.0,
                            op0=ALU.add, op1=ALU.max)
                    for li in range(ni):
                        trash_p = cvt.tile([128, NPATCH], BF16, tag="trash_p",
                                           name="trash_p")
                        nc.gpsimd.scalar_tensor_tensor(
                            out=trash_p,
                            in0=relu_p[:, li * NPATCH:(li + 1) * NPATCH],
                            scalar=0.0, in1=zeros_bf, op0=ALU.add, op1=ALU.add,
                            accum_out=self.pool_p[li % 2][:, col0 + li:
                                                          col0 + li + 1])
                elif jt == "df":
                    for li in range(ni):
                        trash_d = cvt.tile([128, NPATCH], BF16, tag="trash_d",
                                           name="trash_d")
                        nc.vector.scalar_tensor_tensor(
                            out=trash_d,
                            in0=psm[:, li * NPATCH:(li + 1) * NPATCH],
                            scalar=self.b_cnn, in1=zeros_bf,
                            op0=ALU.add, op1=ALU.max,
                            accum_out=self.pool_d[li % 2][:, col0 + li:
                                                          col0 + li + 1])
                else:
                    for li in range(ni):
                        trash_a = cvt.tile([128, NPATCH], F32, tag="trash_a",
                                           name="trash_a")
                        nc.scalar.activation(
                            out=trash_a,
                            in_=psm[:, li * NPATCH:(li + 1) * NPATCH],
                            func=AF.Relu, bias=self.b_cnn, scale=1.0,
                            accum_out=self.pool_a[:, col0 + li:col0 + li + 1])
                ninst += ni

        pool_bf = cvp.tile([128, SB * SPE_PAD], BF16, name="pool_bf")
        nc.vector.tensor_tensor(out=self.pool_d[0], in0=self.pool_d[0],
                                in1=self.pool_d[1], op=ALU.add)
        nc.gpsimd.tensor_tensor(out=self.pool_p[0], in0=self.pool_p[0],
                                in1=self.pool_p[1], op=ALU.add)
        nc.vector.tensor_tensor(out=self.pool_d[0], in0=self.pool_d[0],
                                in1=self.pool_a, op=ALU.add)
        nc.vector.tensor_tensor(out=pool_bf, in0=self.pool_d[0],
                                in1=self.pool_p[0], op=ALU.add)
        self.dbg_dump_tile(10, pool_bf)
        for g in range(4):
            psm = cps.tile([128, DIM], F32, tag="spaps", bufs=2, name="psm_fc")
            nc.tensor.matmul(psm, pool_bf[:, 128 * g:128 * (g + 1)], self.w_fc,
                             start=True, stop=True)
            tmpf = cvt.tile([128, DIM], BF16, tag="fctmp", name="tmp_fc")
            nc.scalar.activation(out=tmpf, in_=psm, func=AF.Relu)
            if self.cfg["use_fc_b"]:
                nc.vector.tensor_tensor(
                    out=tmpf, in0=tmpf,
                    in1=self.bias["fc_b"][0:1, :].to_broadcast((1, DIM)),
                    op=ALU.add)
            nc.gpsimd.tensor_tensor(out=X[:, NT_SPA + g, :], in0=tmpf,
                                    in1=self.pos_spe, op=ALU.add)
        for k in range(4):
            nc.vector.tensor_copy(
                out=X[32 * k:32 * k + 1, NT_SPA:NT, :],
                in_=self.cls2[0:1, 1:2, :].to_broadcast((1, 4, DIM)))
        for g in range(4):
            self.emit_stats(sA0, NT_SPA + g)
        self.flush_stats(sA0, 24, NT)

    # ------------------------------------------------------------ layernorm
    def emit_stats(self, stats, c):
        """bn_stats+aggr for chunk c — emitted opportunistically right after
        the chunk's last residual write, so the 36-chunk serial DVE stats
        chain overlaps the previous phase instead of stalling block start."""
        nc = self.nc
        st = self.stp.tile([128, 6], F32, tag="lnst", name="st")
        nc.vector.bn_stats(out=st, in_=self.X[:, c, :])
        nc.vector.bn_aggr(out=stats[:, c, :], in_=st)

    def flush_stats(self, stats, c0, c1):
        """Finalize chunk range [c0, c1): invstd via DVE fast-rsqrt, then
        mean slot -> -mean*invstd so LN apply is x*inv + nmi (expressible
        on DVE/GpSimd and as an Act Identity activation)."""
        nc = self.nc
        self.rsqrt(stats[:, c0:c1, 1:2], c1 - c0)
        nc.vector.tensor_tensor(out=stats[:, c0:c1, 0:1],
                                in0=stats[:, c0:c1, 0:1],
                                in1=stats[:, c0:c1, 1:2], op=ALU.mult)
        nc.vector.tensor_scalar(out=stats[:, c0:c1, 0:1],
                                in0=stats[:, c0:c1, 0:1],
                                scalar1=-1.0, scalar2=None, op0=ALU.mult)

    def ln_stats(self, stats, c0, c1, grp=12):
        for c in range(c0, c1):
            self.emit_stats(stats, c)
            if c % grp == grp - 1 or c == c1 - 1:
                self.flush_stats(stats, max(c0, c - c % grp), c + 1)

    def rsqrt(self, v, n):
        """v (f32 AP [128, n]) <- 1/sqrt(v + EPS), entirely on DVE
        (fast-inverse-sqrt seed + 2 Newton steps; keeps Ln/Exp off the Act
        engine, whose table reloads thrashed against Gelu/Exp)."""
        nc = self.nc
        p = v.shape[0]
        ve = self.stp.tile([128, 16], F32, tag="rsq_ve", name="rsq_ve")
        y = self.stp.tile([128, 16], F32, tag="rsq_y", name="rsq_y")
        t = self.stp.tile([128, 16], F32, tag="rsq_t", name="rsq_t")
        vn, yn, tn = ve[:p, :n], y[:p, :n], t[:p, :n]
        nc.vector.tensor_scalar(out=vn, in0=v, scalar1=EPS, scalar2=None,
                                op0=ALU.add)
        # y0 = bitcast(0x5f3759df - (bits(ve) >> 1)); C - x == (~x) + (C+1)
        nc.vector.tensor_scalar(out=yn.bitcast(mybir.dt.int32),
                                in0=vn.bitcast(mybir.dt.int32),
                                scalar1=1, scalar2=None,
                                op0=ALU.logical_shift_right)
        nc.vector.tensor_scalar(out=yn.bitcast(mybir.dt.int32),
                                in0=yn.bitcast(mybir.dt.int32),
                                scalar1=0xFFFFFFFF, scalar2=None,
                                op0=ALU.bitwise_xor)
        nc.vector.tensor_scalar(out=yn.bitcast(mybir.dt.int32),
                                in0=yn.bitcast(mybir.dt.int32),
                                scalar1=0x5f3759df + 1, scalar2=None,
                                op0=ALU.add)
        for _ in range(2):
            nc.vector.tensor_tensor(out=tn, in0=yn, in1=yn, op=ALU.mult)
            nc.vector.tensor_tensor(out=tn, in0=tn, in1=vn, op=ALU.mult)
            nc.vector.tensor_scalar(out=tn, in0=tn, scalar1=-0.5,
                                    scalar2=1.5, op0=ALU.mult, op1=ALU.add)
            nc.vector.tensor_tensor(out=yn, in0=yn, in1=tn, op=ALU.mult)
        nc.vector.tensor_copy(out=v, in_=yn)

    def ln_apply_T(self, stats, c, dst, dst_col, affine=None):
        """LN chunk c -> DMA-transpose -> dst[:, :, dst_col:+128] (bf16).
        XBAR mapping: dst[q, e, dst_col+t] = lno[t, 128e+q] — matches the
        _pack_kT weight convention, so no repacking is needed.
        SBUF->SBUF, so 1 in 3 applies runs on the otherwise-idle GpSimd."""
        nc, X = self.nc, self.X
        lno = self.sp.tile([128, DIM], BF16, tag="lno", name="lno")
        r = c % 3
        if r == 2:
            nc.scalar.activation(out=lno, in_=X[:, c, :], func=AF.Identity,
                                 scale=stats[:, c, 1:2],
                                 bias=stats[:, c, 0:1])
        else:
            eng = nc.gpsimd if r == 1 else nc.vector
            eng.tensor_scalar(out=lno, in0=X[:, c, :],
                              scalar1=stats[:, c, 1:2],
                              scalar2=stats[:, c, 0:1],
                              op0=ALU.mult, op1=ALU.add)
        if affine is not None:
            nc.vector.tensor_tensor(out=lno, in0=lno,
                                    in1=affine[0:1, :].to_broadcast((1, DIM)),
                                    op=ALU.mult)
            nc.vector.tensor_tensor(out=lno, in0=lno,
                                    in1=affine[1:2, :].to_broadcast((1, DIM)),
                                    op=ALU.add)
        nc.sync.dma_start_transpose(
            out=dst[:, :, dst_col:dst_col + 128], in_=lno)

    # ------------------------------------------------------------ block
    def qkv_group(self, blk, xlnT_g, q_g, k_g, v_g):
        nc = self.nc
        wqk, wv = self.wblk[f"wqk{blk}"], self.wblk[f"wv{blk}"]
        qkv_b = self.bias.get("qkv_b")
        for m in range(8):
            psm = self.ps_mm()
            for kc in range(4):
                nc.tensor.matmul(psm, wqk[:, kc, 128 * m:128 * (m + 1)],
                                 xlnT_g[:, kc, :], start=(kc == 0),
                                 stop=(kc == 3))
            dst = q_g[:, m, :] if m < 4 else k_g[:, m - 4, :]
            if m < 4:
                if qkv_b is not None:
                    nc.vector.tensor_scalar(out=dst, in0=psm,
                                            scalar1=qkv_b[blk][:, m:m + 1],
                                            scalar2=float(HD) ** -0.5,
                                            op0=ALU.add, op1=ALU.mult)
                elif m % 2 == 0:
                    nc.vector.tensor_scalar(out=dst, in0=psm,
                                            scalar1=float(HD) ** -0.5,
                                            scalar2=None, op0=ALU.mult)
                else:
                    nc.scalar.activation(out=dst, in_=psm, func=AF.Copy,
                                         scale=float(HD) ** -0.5)
            else:
                if qkv_b is not None:
                    nc.vector.tensor_scalar(out=dst, in0=psm,
                                            scalar1=qkv_b[blk][:, m:m + 1],
                                            scalar2=None, op0=ALU.add)
                elif m % 2 == 0:
                    nc.vector.tensor_copy(out=dst, in_=psm)
                else:
                    nc.scalar.activation(out=dst, in_=psm, func=AF.Copy,
                                         scale=1.0)
        for t in range(4):
            psm = self.ps_mm()
            for kc in range(4):
                nc.tensor.matmul(psm, xlnT_g[:, kc, 128 * t:128 * (t + 1)],
                                 wv[:, kc, :], start=(kc == 0), stop=(kc == 3))
            if t % 2 == 0:
                nc.vector.tensor_copy(out=v_g[:, t, :, 0:64], in_=psm)
            else:
                nc.scalar.activation(out=v_g[:, t, :, 0:64], in_=psm,
                                     func=AF.Copy, scale=1.0)
        nc.vector.memset(v_g[:, :, :, 64:65], 1.0)

    def attn_sample(self, q_g, k_g, v_g, oT_g, n0, nw, mchunks, nchunks):
        """mchunks: [(tile, base, rows)]; nchunks: [(col0, rows)]."""
        nc = self.nc
        o_sb = self.ap.tile([128, 2, DIM], BF16, tag="osb", name="o_sb")
        for og in range(2):
            o_ps = [self.ps_o() for _ in nchunks]
            for hh in range(4):
                h = 4 * og + hh
                hp, hc = 64 * (h % 2), h // 2
                aTx = self.ap.tile([128, 2, 256], BF16, tag="aTx", name="aTx")
                if len(mchunks) == 2 and nw == 256:
                    # both m-chunks into one PSUM bank -> single exp op
                    # (chunk1 rows mk..127 hold stale data; excluded by the
                    # K-slice of the o-matmul, so exp of them is harmless)
                    pss = self.ps_sT()
                    for mi, (mt, mb, mk) in enumerate(mchunks):
                        nc.tensor.matmul(
                            pss[:mk, 256 * mi:256 * mi + 256],
                            k_g[hp:hp + 64, hc,
                                128 * mt + mb:128 * mt + mb + mk],
                            q_g[hp:hp + 64, hc, n0:n0 + nw],
                            start=True, stop=True)
                    nc.scalar.activation(
                        out=aTx.rearrange("p a b -> p (a b)"),
                        in_=pss, func=AF.Exp)
                else:
                    for mi, (mt, mb, mk) in enumerate(mchunks):
                        pss = self.ps_sT()
                        nc.tensor.matmul(
                            pss[:mk, :nw],
                            k_g[hp:hp + 64, hc,
                                128 * mt + mb:128 * mt + mb + mk],
                            q_g[hp:hp + 64, hc, n0:n0 + nw],
                            start=True, stop=True)
                        nc.scalar.activation(out=aTx[mb:mb + mk, mi, :nw],
                                             in_=pss[:mk, :nw], func=AF.Exp)
                for ni, (nc0, nr) in enumerate(nchunks):
                    for mi, (mt, mb, mk) in enumerate(mchunks):
                        nc.tensor.matmul(
                            o_ps[ni][:nr, 65 * hh:65 * hh + 65],
                            aTx[mb:mb + mk, mi, nc0:nc0 + nr],
                            v_g[mb:mb + mk, mt, h, :],
                            start=(mi == 0), stop=(mi == len(mchunks) - 1),
                            tile_position=(mb if mk <= 32 else 0, 0))
            for ni, (nc0, nr) in enumerate(nchunks):
                rinv = self.sp.tile([128, 8], F32, tag="rinv", name="rinv")
                nc.vector.reciprocal(out=rinv[:nr, 4 * og:4 * og + 4],
                                     in_=o_ps[ni][:nr, 64:260:65])
                # one op for all 4 heads: broadcast 1/rowsum over head dim
                src = o_ps[ni][:nr, :260].rearrange(
                    "p (h e) -> p h e", e=65)[:, :, 0:64]
                dst = o_sb[:nr, ni, 256 * og:256 * og + 256].rearrange(
                    "p (h e) -> p h e", e=64)
                nc.vector.tensor_tensor(
                    out=dst, in0=src,
                    in1=rinv[:nr, 4 * og:4 * og + 4, None].to_broadcast(
                        (nr, 4, 64)),
                    op=ALU.mult)
        # transpose o (token-major) -> oT_g feature-major columns
        for ni, (nc0, nr) in enumerate(nchunks):
            nc.sync.dma_start_transpose(
                out=oT_g[:, :, n0 + nc0:n0 + nc0 + nr],
                in_=o_sb[:nr, ni, :])

    def proj_group(self, blk, oT_g, base_tile):
        """Blanket residual: X[:, tile, :] += proj(oT). cls rows get the
        wrong (own-stream) delta here; fixed afterwards via cls_fix()."""
        nc, X = self.nc, self.X
        wproj = self.wblk[f"wproj{blk}"]
        for cc in range(4):
            psz = self.ps_mm()
            for e in range(4):
                nc.tensor.matmul(psz, oT_g[:, e, 128 * cc:128 * (cc + 1)],
                                 wproj[:, e, :], start=(e == 0), stop=(e == 3))
            if self.cfg["use_proj_b"]:
                nc.vector.tensor_tensor(
                    out=psz, in0=psz,
                    in1=self.bias["proj_b"][blk:blk + 1, :].to_broadcast((1, DIM)),
                    op=ALU.add)
            dt = base_tile + cc
            nc.vector.tensor_tensor(out=X[:, dt, :], in0=X[:, dt, :],
                                    in1=psz, op=ALU.add)

    def gather_cls(self, tx, ty):
        """DMA-gather the 32 cls rows of X into [16,512] tiles (spa, spe)."""
        nc, X = self.nc, self.X
        nc.sync.dma_start(out=tx, in_=X[0:1, 0:NT_SPA:2, :])
        for k in range(4):
            nc.sync.dma_start(out=ty[k:16:4, :],
                              in_=X[32 * k:32 * k + 1, NT_SPA:NT, :])

    def scatter_cls(self, tx, ty):
        nc, X = self.nc, self.X
        nc.sync.dma_start(out=X[0:1, 0:NT_SPA:2, :], in_=tx)
        for k in range(4):
            nc.sync.dma_start(out=X[32 * k:32 * k + 1, NT_SPA:NT, :],
                              in_=ty[k:16:4, :])

    def block(self, blk):
        nc, X, tc = self.nc, self.X, self.tc
        cfg = self.cfg
        # per-block weights
        self.wblk = {}
        for nm in ("wqk", "wv", "wproj", "wfc1", "wfc2"):
            key = nm + str(blk)
            shape = [128, 4, 2 * DIM] if nm == "wqk" else [128, 4, DIM]
            t = self.wp.tile(shape, BF16, tag=nm,
                             bufs=2 if nm == "wv" else 1, name="w_" + key)
            nc.sync.dma_start(out=t, in_=self.P[key][:])
            self.wblk[key] = t

        aff1 = self.bias["n1_wb"][blk] if cfg["use_n1"] else None
        stats1 = self.stats[f"A{blk}"]     # prefilled during previous phase
        stats2 = self.stats[f"M{blk}"]
        # snapshot cls rows before any residual update (for the cls fix)
        xc0_x = self.bp.tile([16, DIM], F32, tag="xc0x", name="xc0_x")
        xc0_y = self.bp.tile([16, DIM], F32, tag="xc0y", name="xc0_y")
        self.gather_cls(xc0_x, xc0_y)

        # LN the 16 cls tokens of each stream directly from the gathered
        # rows (per-token stats fetched from stats1), transpose once, and
        # scatter the columns into the OTHER stream's xlnT tiles.  This
        # replaces the column-swap dance and removes the dependency of the
        # spe stream on all spa LN groups (and vice versa).
        stx = self.bp.tile([16, 2], F32, tag="stx", name="stx")
        nc.sync.dma_start(out=stx, in_=stats1[0:1, 0:NT_SPA:2, :])
        sty = self.bp.tile([16, 2], F32, tag="sty", name="sty")
        for k in range(4):
            nc.sync.dma_start(out=sty[k:16:4, :],
                              in_=stats1[32 * k:32 * k + 1, NT_SPA:NT, :])
        clsln = []
        for nm, xc, st in (("x", xc0_x, stx), ("y", xc0_y, sty)):
            cl = self.bp.tile([16, DIM], BF16, tag=f"cls{nm}", name=f"cls{nm}")
            nc.vector.tensor_scalar(out=cl, in0=xc, scalar1=st[:, 1:2],
                                    scalar2=st[:, 0:1],
                                    op0=ALU.mult, op1=ALU.add)
            if aff1 is not None:
                nc.vector.tensor_tensor(out=cl, in0=cl,
                                        in1=aff1[0:1, :].to_broadcast((1, DIM)),
                                        op=ALU.mult)
                nc.vector.tensor_tensor(out=cl, in0=cl,
                                        in1=aff1[1:2, :].to_broadcast((1, DIM)),
                                        op=ALU.add)
            clT = self.bp.tile([128, 4, 16], BF16, tag=f"cls{nm}T",
                               name=f"cls{nm}T")
            nc.sync.dma_start_transpose(out=clT, in_=cl)
            clsln.append(clT)
        clsxT, clsyT = clsln

        # Pre-double the cls rows: the blanket residual with SWAPPED oT
        # columns then produces exactly 2*x_cls + proj(other-stream cls out),
        # eliminating the old gather/fix/scatter sync between attention and
        # MLP.  (xlnT never reads X cls rows — their columns are overwritten
        # by the cls scatters — and stats1/xc0 were captured above.)
        nc.vector.tensor_scalar(out=X[0:1, 0:NT_SPA:2, :],
                                in0=X[0:1, 0:NT_SPA:2, :], scalar1=2.0,
                                scalar2=None, op0=ALU.mult)
        for k in range(4):
            nc.vector.tensor_scalar(out=X[32 * k:32 * k + 1, NT_SPA:NT, :],
                                    in0=X[32 * k:32 * k + 1, NT_SPA:NT, :],
                                    scalar1=2.0, scalar2=None, op0=ALU.mult)

        xlnT_spe = self.bp.tile([128, 4, DIM], BF16, tag="xlnTspe",
                                name="xlnT_spe")
        for lc in range(4):
            self.ln_apply_T(stats1, NT_SPA + lc, xlnT_spe, 128 * lc, aff1)
        nc.gpsimd.tensor_copy(out=xlnT_spe[:, :, 0:SB * 32:32], in_=clsxT)

        q_s = self.gp.tile([128, 4, DIM], BF16, tag="qs", bufs=1, name="q_s")
        k_s = self.gp.tile([128, 4, DIM], BF16, tag="ks", bufs=1, name="k_s")
        v_s = self.gp.tile([128, 4, NH, 65], BF16, tag="vs", bufs=1,
                           name="v_s")
        oT_s = self.gp.tile([128, 4, DIM], BF16, tag="oTs", bufs=1,
                            name="oT_s")
        # spe attn samples 2g, 2g+1 run inside spa group g: the cls-column
        # swap before proj g needs them
        spe_sched = {g: (2 * g, 2 * g + 2) for g in range(8)}

        spa_nchunks = [(0, 128), (128, 128)]
        for g in range(8):
            xlnT_g = self.gp.tile([128, 4, DIM], BF16, tag="xlnT", name="xlnT_g")
            for lc in range(4):
                self.ln_apply_T(stats1, 4 * g + lc, xlnT_g, 128 * lc, aff1)
            nc.gpsimd.tensor_copy(out=xlnT_g[:, :, 0:DIM:256],
                                  in_=clsyT[:, :, 2 * g:2 * g + 2])
            q_g = self.gp.tile([128, 4, DIM], BF16, tag="qg", name="q_g")
            k_g = self.gp.tile([128, 4, DIM], BF16, tag="kg", name="k_g")
            v_g = self.gp.tile([128, 4, NH, 65], BF16, tag="vg", name="v_g")
            self.qkv_group(blk, xlnT_g, q_g, k_g, v_g)
            if g == 0:
                self.qkv_group(blk, xlnT_spe, q_s, k_s, v_s)
            for s in range(*spe_sched.get(g, (0, 0))):
                self.attn_sample(q_s, k_s, v_s, oT_s, 32 * s, 32,
                                 [(s // 4, 32 * (s % 4), 31)], [(0, 32)])
            oT_g = self.gp.tile([128, 4, DIM], BF16, tag="oTg", name="oT_g")
            for j in range(2):
                self.attn_sample(q_g, k_g, v_g, oT_g, 256 * j, 256,
                                 [(2 * j, 0, 128), (2 * j + 1, 0, 98)],
                                 spa_nchunks)
            # swap the two streams' cls attention-output columns pre-proj
            for j in range(2):
                s = 2 * g + j
                ctm = self.sp.tile([128, 4, 1], BF16, tag="ctm", name="ctm")
                nc.gpsimd.tensor_copy(out=ctm,
                                      in_=oT_s[:, :, 32 * s:32 * s + 1])
                nc.gpsimd.tensor_copy(out=oT_s[:, :, 32 * s:32 * s + 1],
                                      in_=oT_g[:, :, 256 * j:256 * j + 1])
                nc.gpsimd.tensor_copy(out=oT_g[:, :, 256 * j:256 * j + 1],
                                      in_=ctm)
            self.proj_group(blk, oT_g, 4 * g)
            for c in range(4 * g, 4 * g + 4):
                self.emit_stats(stats2, c)
            self.flush_stats(stats2, 4 * g, 4 * g + 4)
            # one-group delay: group g-1's stats flushed last iteration, so
            # this MLP never stalls an in-order queue mid-attention
            if g >= 2 and g % 2 == 0:
                self.mlp_group(blk, stats2, g - 2)
                self.mlp_group(blk, stats2, g - 1)

        # spe stream tail: proj with fully-swapped oT_s, then trailing MLPs
        self.proj_group(blk, oT_s, NT_SPA)
        for c in range(NT_SPA, NT):
            self.emit_stats(stats2, c)
        self.flush_stats(stats2, NT_SPA, NT)
        self.mlp_group(blk, stats2, 6)
        self.mlp_group(blk, stats2, 7)
        self.mlp_group(blk, stats2, 8)

    def mlp_group(self, blk, stats2, r):
        nc, X, cfg = self.nc, self.X, self.cfg
        wfc1, wfc2 = self.wblk[f"wfc1{blk}"], self.wblk[f"wfc2{blk}"]
        aff2 = self.bias["n2_wb"][blk] if cfg["use_n2"] else None
        fc1_b = self.bias.get("fc1_b")
        x2T = self.gp.tile([128, 4, DIM], BF16, tag="x2T", name="x2T")
        for lc in range(4):
            self.ln_apply_T(stats2, 4 * r + lc, x2T, 128 * lc, aff2)
        h1T = self.gp.tile([128, 4, DIM], BF16, tag="h1T", name="h1T")
        for m in range(4):
            psm = self.ps_mm()
            for kc in range(4):
                nc.tensor.matmul(psm, wfc1[:, kc, 128 * m:128 * (m + 1)],
                                 x2T[:, kc, :], start=(kc == 0),
                                 stop=(kc == 3))
            b1 = fc1_b[blk][:, m:m + 1] if fc1_b is not None else 0.0
            nc.scalar.activation(out=h1T[:, m, :], in_=psm, func=AF.Gelu,
                                 bias=b1, scale=1.0)
        for cc in range(4):
            psm = self.ps_mm()
            for kc in range(4):
                nc.tensor.matmul(psm, h1T[:, kc, 128 * cc:128 * (cc + 1)],
                                 wfc2[:, kc, :], start=(kc == 0),
                                 stop=(kc == 3))
            if cfg["use_fc2_b"]:
                nc.vector.tensor_tensor(
                    out=psm, in0=psm,
                    in1=self.bias["fc2_b"][blk:blk + 1, :].to_broadcast(
                        (1, DIM)), op=ALU.add)
            dt = 4 * r + cc
            nc.vector.tensor_tensor(out=X[:, dt, :], in0=X[:, dt, :],
                                    in1=psm, op=ALU.add)
        if blk == 0:
            sA1 = self.stats["A1"]
            for c in range(4 * r, 4 * r + 4):
                self.emit_stats(sA1, c)
            if r % 3 == 2:
                self.flush_stats(sA1, 12 * (r // 3), 12 * (r // 3) + 12)

    # ------------------------------------------------------------ head
    def head(self):
        nc, X, cfg = self.nc, self.X, self.cfg
        xcf = [self.bp.tile([16, DIM], F32, tag=f"xcf{h}", name=f"xcf{h}")
               for h in range(2)]
        self.gather_cls(xcf[0], xcf[1])
        clsn = [self.bp.tile([16, DIM], BF16, tag=f"clsn{h}", name=f"clsn{h}")
                for h in range(2)]
        for half in range(2):
            st = self.sp.tile([16, 6], F32, tag="lnstf", name="stf")
            mv = self.sp.tile([16, 2], F32, tag="fmv", name="mvf")
            nc.vector.bn_stats(out=st, in_=xcf[half])
            nc.vector.bn_aggr(out=mv, in_=st)
            self.rsqrt(mv[:, 1:2], 1)
            nc.vector.tensor_scalar(out=clsn[half], in0=xcf[half],
                                    scalar1=mv[:, 0:1], scalar2=mv[:, 1:2],
                                    op0=ALU.subtract, op1=ALU.mult)
            if cfg["use_nf"]:
                nc.vector.tensor_tensor(
                    out=clsn[half], in0=clsn[half],
                    in1=self.bias["nf_w"][half:half + 1, :].to_broadcast((1, DIM)),
                    op=ALU.mult)
                nc.vector.tensor_tensor(
                    out=clsn[half], in0=clsn[half],
                    in1=self.bias["nf_b"][half:half + 1, :].to_broadcast((1, DIM)),
                    op=ALU.add)
        clsT = self.bp.tile([128, 8, NCLS], BF16, tag="clsT", name="clsT")
        for half in range(2):
            nc.sync.dma_start_transpose(
                out=clsT[:, 4 * half:4 * half + 4, :], in_=clsn[half])
        psh = self.ps.tile([16, NCLS], F32, tag="mm", bufs=3, name="psh")
        for dc in range(8):
            nc.tensor.matmul(psh, clsT[:, dc, :], self.whead[:, dc, :],
                             start=(dc == 0), stop=(dc == 7))
        out_sb = self.bp.tile([16, NCLS], F32, tag="out_sb", name="out_sb")
        if cfg["use_head_b"]:
            nc.vector.tensor_tensor(
                out=out_sb, in0=psh,
                in1=self.bias["head_b"][0:1, :].to_broadcast((1, NCLS)),
                op=ALU.add)
        else:
            nc.vector.tensor_copy(out=out_sb, in_=psh)
        nc.sync.dma_start(out=self.out_p[:], in_=out_sb)


# ---------------------------------------------------------------- entry
def kernel(**inputs):
    w = prepare_weights(inputs)
    nc = build_program(w)
    in_maps = make_in_maps(inputs, w)
    res = run_bass_kernel_spmd(nc, in_maps, list(range(NCORES)))
    out = np.concatenate([np.asarray(r["out"], np.float32)
                          for r in res.results], axis=0)
    return out


def run_traced(inputs):
    """For test.py: returns (out, BassKernelResults with exec_time_ns)."""
    w = prepare_weights(inputs)
    nc = build_program(w)
    in_maps = make_in_maps(inputs, w)
    res = run_bass_kernel_spmd(nc, in_maps, list(range(NCORES)), trace=True)
    out = np.concatenate([np.asarray(r["out"], np.float32)
                          for r in res.results], axis=0)
    return out, res

